# revision 6
# baseline (speedup 1.0000x reference)
"""HMLC loss kernel for 8 Trainium2 NeuronCores (Bass/Tile).

Strategy (queue-sharded data parallelism):
  * All mask/dedup/queue-evolution logic in the reference depends ONLY on the
    integer labels -> computed exactly on host (numpy).
  * The queue (32768 cols) is split into 32 shards (8 cores x 4 vshards).
    Within each shard, columns are ordered by "lifetime" (the last level at
    which the column is still active), so the active set at every level is a
    prefix. The assignment is round-robin over the lifetime-sorted global
    column order, so prefix lengths differ by at most 1 across shards and a
    single compiled SPMD program (prefix bounds baked as max over shards)
    serves all cores; a <=1-column-wide additive -6e4 mask (per-core DATA)
    handles the remainder.
  * Device per (vshard, anchor-chunk): PE computes sim = (f/TEMP) @ fq_shard.T
    into PSUM [128,1024]; per level the stats are
        neg_lm  = -max(sim[:, :n])                  (VectorE tensor_reduce)
        denom   = sum exp(sim[:, :n] + neg_lm)      (ScalarE activation+accum)
        possum  = sum (kq==ka) * sim[:, :n]         (VectorE scalar_tensor_tensor+accum)
  * Host merges the 32 shards per level (online softmax) in float64 and runs
    the scalar hmce chain.
"""

import os
import sys
import time
from contextlib import ExitStack

if "/opt/trn_rl_repo" not in sys.path:
    sys.path.insert(0, "/opt/trn_rl_repo")

import numpy as np

import concourse.bass as bass  # noqa: E402
import concourse.bacc as bacc  # noqa: E402
import concourse.tile as tile  # noqa: E402
from concourse import mybir  # noqa: E402
from concourse.bass_utils import run_bass_kernel_spmd  # noqa: E402

TEMP = 0.07
BASE_TEMP = 0.07
NCORES = 8
NVS = 4          # vshards per core
P = 128          # partitions
MASK_VAL = -60000.0

# matmul precision mode: "f32" (exact, 4 cyc/row), "f32r" (1 cyc/row),
# "bf16x3" (hi/lo split, 3 passes, 1 cyc/row each)
MM_MODE = os.environ.get("HMLC_MM_MODE", "f32")

# populated by kernel() for test harness introspection
LAST_RUN = {}


# ---------------------------------------------------------------- host masks
def _host_masks(labels, labels_queue):
    """Exact replication of the reference's label-only mask evolution."""
    B, L = labels.shape
    Q = labels_queue.shape[0]
    base = int(max(labels.max(), labels_queue.max())) + 1
    pw = base ** np.arange(L - 1, -1, -1)

    anchor_active = np.ones(B, bool)
    queue_active = np.ones(Q, bool)
    order = np.arange(B)

    levels = []
    for l in range(1, L):
        ncols = L - l
        w = (pw * (np.arange(L) < ncols)).astype(np.int64)
        ka = labels.astype(np.int64) @ w
        kq = labels_queue.astype(np.int64) @ w
        maxk = int(max(ka.max(), kq.max())) + 1
        bc = np.bincount(kq[queue_active], minlength=maxk)
        cnt = np.where(anchor_active, bc[ka], 0)
        pres = np.zeros(maxk, bool)
        pres[ka[anchor_active]] = True
        newmatch = queue_active & pres[kq]
        levels.append(dict(
            ka=ka.copy(), kq=kq.copy(),
            queue_active=queue_active.copy(),
            cnt=cnt.copy(),
        ))
        same = (ka[:, None] == ka[None, :]) & anchor_active[:, None] & anchor_active[None, :]
        max_ord = np.max(np.where(same, order[None, :], -1), axis=1)
        kept = anchor_active & (order == max_ord)
        rank = (kept[None, :] & (ka[None, :] < ka[:, None])).sum(1)
        order = np.where(kept, rank, -1)
        anchor_active = kept
        queue_active = queue_active & ~newmatch
    return levels


# ------------------------------------------------------------ device program
def _build_program(D, B, CQ, nmx, nmn, mm_mode):
    NLEV = 3
    f32 = mybir.dt.float32
    NB = B // P       # anchor chunks
    NK = D // P       # contraction chunks
    QS = CQ // NVS    # vshard width

    nc = bacc.Bacc("TRN2", target_bir_lowering=False, debug=False)

    two_pass = mm_mode == "bf16x3"
    if two_pass:
        bf16 = mybir.dt.bfloat16
        ft_hi_d = nc.dram_tensor("ft_hi", [D, B], bf16, kind="ExternalInput").ap()
        ft_lo_d = nc.dram_tensor("ft_lo", [D, B], bf16, kind="ExternalInput").ap()
        fqt_hi_d = nc.dram_tensor("fqt_hi", [D, CQ], bf16, kind="ExternalInput").ap()
        fqt_lo_d = nc.dram_tensor("fqt_lo", [D, CQ], bf16, kind="ExternalInput").ap()
    else:
        ft_d = nc.dram_tensor("ft", [D, B], f32, kind="ExternalInput").ap()
        fqt_d = nc.dram_tensor("fqt", [D, CQ], f32, kind="ExternalInput").ap()
    kq_d = nc.dram_tensor("kq", [NLEV, CQ], f32, kind="ExternalInput").ap()
    ka_d = nc.dram_tensor("ka", [NLEV, P, NB], f32, kind="ExternalInput").ap()
    madd_d = nc.dram_tensor("madd", [NVS, NLEV, 1], f32, kind="ExternalInput").ap()
    stats_d = nc.dram_tensor(
        "stats", [NVS, NLEV, 3, P, NB], f32, kind="ExternalOutput").ap()

    with tile.TileContext(nc) as tc, ExitStack() as ctx:
        const_pool = ctx.enter_context(tc.tile_pool(name="const", bufs=1))
        fqt_pool = ctx.enter_context(tc.tile_pool(name="fqt", bufs=2))
        kq_pool = ctx.enter_context(tc.tile_pool(name="kqb", bufs=2))
        scr_pool = ctx.enter_context(tc.tile_pool(name="scr", bufs=4))
        st_pool = ctx.enter_context(tc.tile_pool(name="st", bufs=2))
        psum_pool = ctx.enter_context(tc.tile_pool(name="ps", bufs=3, space="PSUM"))

        if two_pass:
            ft_hi = const_pool.tile([P, NK, B], bf16)
            nc.sync.dma_start(out=ft_hi, in_=ft_hi_d.rearrange("(k p) b -> p k b", p=P))
            ft_lo = const_pool.tile([P, NK, B], bf16)
            nc.sync.dma_start(out=ft_lo, in_=ft_lo_d.rearrange("(k p) b -> p k b", p=P))
        else:
            ft_sb = const_pool.tile([P, NK, B], f32)
            nc.sync.dma_start(out=ft_sb, in_=ft_d.rearrange("(k p) b -> p k b", p=P))
        ka_sb = const_pool.tile([P, NLEV, NB], f32)
        nc.sync.dma_start(out=ka_sb, in_=ka_d.rearrange("l p c -> p l c"))

        for v in range(NVS):
            if two_pass:
                fqt_hi = fqt_pool.tile([P, NK, QS], bf16, tag="fqt_hi")
                nc.sync.dma_start(
                    out=fqt_hi,
                    in_=fqt_hi_d[:, v * QS:(v + 1) * QS].rearrange("(k p) q -> p k q", p=P))
                fqt_lo = fqt_pool.tile([P, NK, QS], bf16, tag="fqt_lo")
                nc.sync.dma_start(
                    out=fqt_lo,
                    in_=fqt_lo_d[:, v * QS:(v + 1) * QS].rearrange("(k p) q -> p k q", p=P))
            else:
                fqt_sb = fqt_pool.tile([P, NK, QS], f32)
                nc.sync.dma_start(
                    out=fqt_sb,
                    in_=fqt_d[:, v * QS:(v + 1) * QS].rearrange("(k p) q -> p k q", p=P))

            kqb = []
            for li in range(NLEV):
                n = nmx[li]
                if n == 0:
                    kqb.append(None)
                    continue
                t = kq_pool.tile([P, nmx[0]], f32, tag=f"kqb{li}")
                nc.gpsimd.dma_start(
                    out=t[:, :n],
                    in_=kq_d[li:li + 1, v * QS: v * QS + n].to_broadcast([P, n]))
                kqb.append(t)
            mt = {}
            for li in range(1, NLEV):
                w = nmx[li] - nmn[li]
                if nmx[li] > 0 and w > 0:
                    t = kq_pool.tile([P, w], f32, tag=f"madd{li}")
                    nc.gpsimd.dma_start(
                        out=t, in_=madd_d[v, li:li + 1, 0].to_broadcast([P, w]))
                    mt[li] = t

            neglm_t = [st_pool.tile([P, NB], f32, tag=f"nl{li}", name=f"nl{li}_{v}")
                       for li in range(NLEV)]
            den_t = [st_pool.tile([P, NB], f32, tag=f"dn{li}", name=f"dn{li}_{v}")
                     for li in range(NLEV)]
            pos_t = [st_pool.tile([P, NB], f32, tag=f"po{li}", name=f"po{li}_{v}")
                     for li in range(NLEV)]

            for c in range(NB):
                ps = psum_pool.tile([P, QS], f32)
                for g in range(QS // 512):
                    gs = slice(g * 512, (g + 1) * 512)
                    if two_pass:
                        passes = [(ft_hi, fqt_hi), (ft_hi, fqt_lo), (ft_lo, fqt_hi)]
                        n_mm = len(passes) * NK
                        i = 0
                        for lt, rt in passes:
                            for k in range(NK):
                                nc.tensor.matmul(
                                    ps[:, gs],
                                    lt[:, k, c * P:(c + 1) * P],
                                    rt[:, k, gs],
                                    start=(i == 0), stop=(i == n_mm - 1))
                                i += 1
                    else:
                        for k in range(NK):
                            lhsT = ft_sb[:, k, c * P:(c + 1) * P]
                            rhs = fqt_sb[:, k, gs]
                            if mm_mode == "f32r":
                                lhsT = lhsT.bitcast(mybir.dt.float32r)
                                rhs = rhs.bitcast(mybir.dt.float32r)
                            nc.tensor.matmul(
                                ps[:, gs], lhsT, rhs,
                                start=(k == 0), stop=(k == NK - 1))

                for li in range(3):
                    n = nmx[li]
                    if n == 0:
                        continue
                    if li in mt:
                        nc.vector.tensor_add(
                            ps[:, nmn[li]:nmx[li]], ps[:, nmn[li]:nmx[li]], mt[li])
                    nc.vector.tensor_reduce(
                        neglm_t[li][:, c:c + 1], ps[:, :n],
                        axis=mybir.AxisListType.X, op=mybir.AluOpType.max,
                        negate=True)
                    e_scr = scr_pool.tile([P, nmx[0]], f32, tag="escr")
                    nc.scalar.activation(
                        e_scr[:, :n], ps[:, :n],
                        mybir.ActivationFunctionType.Exp,
                        bias=neglm_t[li][:, c:c + 1], scale=1.0,
                        accum_out=den_t[li][:, c:c + 1])
                    m_scr = scr_pool.tile([P, nmx[0]], f32, tag="mscr")
                    nc.vector.scalar_tensor_tensor(
                        out=m_scr[:, :n], in0=kqb[li][:, :n],
                        scalar=ka_sb[:, li, c:c + 1], in1=ps[:, :n],
                        op0=mybir.AluOpType.is_equal, op1=mybir.AluOpType.mult,
                        accum_out=pos_t[li][:, c:c + 1])

            for li in range(NLEV):
                for si, t in ((0, neglm_t[li]), (1, den_t[li]), (2, pos_t[li])):
                    nc.sync.dma_start(out=stats_d[v, li, si], in_=t)

    nc.compile()
    return nc


# ----------------------------------------------------------------- host prep
def _prepare(features, labels, features_queue, labels_queue):
    """Host-side: masks, balanced shard assignment, per-core input arrays."""
    B, D = features.shape
    Q = features_queue.shape[0]
    S = NCORES * NVS
    QS_SHARD = Q // S
    NB = B // P
    NLEV = 3

    levels = _host_masks(labels, labels_queue)

    # lifetime = last level at which a queue column is active (1..3)
    life = np.ones(Q, np.int64)
    for li in (1, 2):
        life += levels[li]["queue_active"].astype(np.int64)
    order_cols = np.argsort(-life, kind="stable")
    perm = order_cols.reshape(QS_SHARD, S).T  # [S, QS_SHARD]: shard s -> cols

    n_per_shard = np.zeros((S, NLEV), np.int64)
    n_per_shard[:, 0] = QS_SHARD
    for li in (1, 2):
        n_per_shard[:, li] = levels[li]["queue_active"][perm].sum(axis=1)
    nmx = [int(n_per_shard[:, li].max()) for li in range(NLEV)]
    nmn = [int(n_per_shard[:, li].min()) for li in range(NLEV)]
    assert nmx[0] == nmn[0] == QS_SHARD
    for li in range(1, NLEV):
        assert nmx[li] - nmn[li] <= 1, (nmx, nmn)

    # ---- per-core input arrays
    ftS = np.ascontiguousarray((features / TEMP).T)  # [D, B]
    fqT = np.ascontiguousarray(features_queue.T)     # [D, Q]

    ka_r = np.empty((NLEV, P, NB), np.float32)
    for li in range(NLEV):
        ka_r[li] = levels[li]["ka"].astype(np.float32).reshape(NB, P).T

    in_maps = []
    for c in range(NCORES):
        cols = perm[c * NVS:(c + 1) * NVS].reshape(-1)  # [CQ]
        fqt_c = np.ascontiguousarray(fqT[:, cols])
        kq_c = np.empty((NLEV, NVS * QS_SHARD), np.float32)
        for li in range(NLEV):
            kq_c[li] = np.where(
                levels[li]["queue_active"][cols],
                levels[li]["kq"][cols].astype(np.float32), np.float32(-1.0))
        madd_c = np.zeros((NVS, NLEV, 1), np.float32)
        for v in range(NVS):
            s = c * NVS + v
            for li in range(1, NLEV):
                if nmx[li] - nmn[li] > 0:
                    # mask the single boundary column if dead for this shard
                    madd_c[v, li, 0] = (
                        np.float32(MASK_VAL)
                        if n_per_shard[s, li] < nmx[li] else np.float32(0.0))
        m = {"kq": kq_c, "ka": ka_r, "madd": madd_c}
        if MM_MODE == "bf16x3":
            import ml_dtypes
            bf = ml_dtypes.bfloat16
            ft_hi = ftS.astype(bf)
            ft_lo = (ftS - ft_hi.astype(np.float32)).astype(bf)
            fq_hi = fqt_c.astype(bf)
            fq_lo = (fqt_c - fq_hi.astype(np.float32)).astype(bf)
            m.update(ft_hi=ft_hi, ft_lo=ft_lo, fqt_hi=fq_hi, fqt_lo=fq_lo)
        else:
            m.update(ft=ftS, fqt=fqt_c)
        in_maps.append(m)

    return dict(in_maps=in_maps, levels=levels, perm=perm,
                n_per_shard=n_per_shard, nmx=nmx, nmn=nmn,
                B=B, D=D, Q=Q, S=S, QS_SHARD=QS_SHARD, NB=NB, NLEV=NLEV)


# -------------------------------------------------------------------- kernel
def kernel(features, labels, features_queue, labels_queue):
    t0 = time.time()
    features = np.asarray(features, dtype=np.float32)
    features_queue = np.asarray(features_queue, dtype=np.float32)
    labels = np.asarray(labels)
    labels_queue = np.asarray(labels_queue)

    prep = _prepare(features, labels, features_queue, labels_queue)
    in_maps = prep["in_maps"]
    levels = prep["levels"]
    n_per_shard = prep["n_per_shard"]
    nmx, nmn = prep["nmx"], prep["nmn"]
    B, D = prep["B"], prep["D"]
    S, QS_SHARD = prep["S"], prep["QS_SHARD"]
    NLEV = prep["NLEV"]
    t_prep = time.time() - t0

    # ---- build + run device program
    t0 = time.time()
    nc = _build_program(D, B, NVS * QS_SHARD, nmx, nmn, MM_MODE)
    t_build = time.time() - t0

    t0 = time.time()
    br = run_bass_kernel_spmd(nc, in_maps, core_ids=list(range(NCORES)))
    t_run = time.time() - t0

    LAST_RUN.clear()
    LAST_RUN.update(
        exec_time_ns=br.exec_time_ns,
        mean_exec_time_ns=getattr(br, "mean_exec_time_ns", None),
        t_prep=t_prep, t_build=t_build, t_run=t_run,
        profile_json=br.profile_json,
        instructions_and_trace=br.instructions_and_trace,
        nmx=nmx, nmn=nmn)

    # ---- host merge (float64)
    t0 = time.time()
    # stats[c]: [NVS, NLEV, 3, P, NB] -> per shard arrays [B]
    neg_lm = np.empty((S, NLEV, B), np.float64)
    den = np.empty((S, NLEV, B), np.float64)
    pos = np.empty((S, NLEV, B), np.float64)
    for c in range(NCORES):
        st = br.results[c]["stats"]  # [NVS, NLEV, 3, P, NB]
        for v in range(NVS):
            s = c * NVS + v
            for li in range(NLEV):
                neg_lm[s, li] = st[v, li, 0].T.reshape(-1)
                den[s, li] = st[v, li, 1].T.reshape(-1)
                pos[s, li] = st[v, li, 2].T.reshape(-1)

    cum = 0.0
    max_lower = -np.inf
    for li in range(NLEV):
        l = li + 1
        cnt = levels[li]["cnt"].astype(np.float64)
        valid = n_per_shard[:, li] > 0  # shards with any columns at this level
        lm_s = -neg_lm[valid, li]      # [S', B]
        den_s = den[valid, li]
        pos_s = pos[valid, li]
        if lm_s.shape[0] == 0:
            layer_loss = 0.0
        else:
            lm = lm_s.max(axis=0)
            dtot = (den_s * np.exp(lm_s - lm[None, :])).sum(axis=0)
            ptot = pos_s.sum(axis=0)
            with np.errstate(divide="ignore", invalid="ignore"):
                mean = (ptot - cnt * (lm + np.log(dtot))) / (cnt + 1e-12)
            mean = np.where(cnt > 0, mean, 0.0)
            loss_i = -(TEMP / BASE_TEMP) * mean
            num = float((cnt > 0).sum())
            layer_loss = float(loss_i.sum() / (num + 1e-12))
        layer_loss = max(max_lower, layer_loss)
        cum = cum + (2.0 ** (1.0 / l)) * layer_loss
        max_lower = max(max_lower, layer_loss)

    LAST_RUN["t_merge"] = time.time() - t0
    return np.float32(cum)


# revision 10
# speedup vs baseline: 1.4510x; 1.4510x over previous
"""HMLC loss kernel for 8 Trainium2 NeuronCores (Bass/Tile).

Strategy (queue-sharded data parallelism):
  * All mask/dedup/queue-evolution logic in the reference depends ONLY on the
    integer labels -> computed exactly on host (numpy).
  * The queue (32768 cols) is split into 32 shards (8 cores x 4 vshards).
    Within each shard, columns are ordered by "lifetime" (the last level at
    which the column is still active), so the active set at every level is a
    prefix. The assignment is round-robin over the lifetime-sorted global
    column order, so prefix lengths differ by at most 1 across shards and a
    single compiled SPMD program (prefix bounds baked as max over shards)
    serves all cores; a <=1-column-wide additive -6e4 mask (per-core DATA)
    handles the remainder.
  * Device per (vshard, anchor-chunk): PE computes sim = (f/TEMP) @ fq_shard.T
    into PSUM [128,1024]; per level the stats are
        neg_lm  = -max(sim[:, :n])                  (VectorE tensor_reduce)
        denom   = sum exp(sim[:, :n] + neg_lm)      (ScalarE activation+accum)
        possum  = sum (kq==ka) * sim[:, :n]         (VectorE scalar_tensor_tensor+accum)
  * Host merges the 32 shards per level (online softmax) in float64 and runs
    the scalar hmce chain.
"""

import os
import sys
import time
from contextlib import ExitStack

if "/opt/trn_rl_repo" not in sys.path:
    sys.path.insert(0, "/opt/trn_rl_repo")

import numpy as np

import concourse.bass as bass  # noqa: E402
import concourse.bacc as bacc  # noqa: E402
import concourse.tile as tile  # noqa: E402
from concourse import mybir  # noqa: E402
from concourse.bass_utils import run_bass_kernel_spmd  # noqa: E402

TEMP = 0.07
BASE_TEMP = 0.07
NCORES = 8
NVS = 4          # vshards per core
P = 128          # partitions
MASK_VAL = -60000.0

# matmul precision mode: "f32" (exact, 4 cyc/row), "f32r" (1 cyc/row),
# "bf16x3" (hi/lo split, 3 passes, 1 cyc/row each)
MM_MODE = os.environ.get("HMLC_MM_MODE", "f32")

# populated by kernel() for test harness introspection
LAST_RUN = {}


# ---------------------------------------------------------------- host masks
def _host_masks(labels, labels_queue):
    """Exact replication of the reference's label-only mask evolution."""
    B, L = labels.shape
    Q = labels_queue.shape[0]
    base = int(max(labels.max(), labels_queue.max())) + 1
    pw = base ** np.arange(L - 1, -1, -1)

    anchor_active = np.ones(B, bool)
    queue_active = np.ones(Q, bool)
    order = np.arange(B)

    levels = []
    for l in range(1, L):
        ncols = L - l
        w = (pw * (np.arange(L) < ncols)).astype(np.int64)
        ka = labels.astype(np.int64) @ w
        kq = labels_queue.astype(np.int64) @ w
        maxk = int(max(ka.max(), kq.max())) + 1
        bc = np.bincount(kq[queue_active], minlength=maxk)
        cnt = np.where(anchor_active, bc[ka], 0)
        pres = np.zeros(maxk, bool)
        pres[ka[anchor_active]] = True
        newmatch = queue_active & pres[kq]
        levels.append(dict(
            ka=ka.copy(), kq=kq.copy(),
            queue_active=queue_active.copy(),
            cnt=cnt.copy(),
        ))
        same = (ka[:, None] == ka[None, :]) & anchor_active[:, None] & anchor_active[None, :]
        max_ord = np.max(np.where(same, order[None, :], -1), axis=1)
        kept = anchor_active & (order == max_ord)
        rank = (kept[None, :] & (ka[None, :] < ka[:, None])).sum(1)
        order = np.where(kept, rank, -1)
        anchor_active = kept
        queue_active = queue_active & ~newmatch
    return levels


# ------------------------------------------------------------ device program
def _build_program(D, B, CQ, nmx, nmn, mm_mode):
    NLEV = 3
    f32 = mybir.dt.float32
    NB = B // P       # anchor chunks
    NK = D // P       # contraction chunks
    QS = CQ // NVS    # vshard width

    nc = bacc.Bacc("TRN2", target_bir_lowering=False, debug=False)

    two_pass = mm_mode == "bf16x3"
    if two_pass:
        bf16 = mybir.dt.bfloat16
        ft_hi_d = nc.dram_tensor("ft_hi", [D, B], bf16, kind="ExternalInput").ap()
        ft_lo_d = nc.dram_tensor("ft_lo", [D, B], bf16, kind="ExternalInput").ap()
        fqt_hi_d = nc.dram_tensor("fqt_hi", [D, CQ], bf16, kind="ExternalInput").ap()
        fqt_lo_d = nc.dram_tensor("fqt_lo", [D, CQ], bf16, kind="ExternalInput").ap()
    else:
        mmdt = mybir.dt.float32r if mm_mode == "f32r" else f32
        ft_d = nc.dram_tensor("ft", [D, B], mmdt, kind="ExternalInput").ap()
        fqt_d = nc.dram_tensor("fqt", [D, CQ], mmdt, kind="ExternalInput").ap()
    kq_d = nc.dram_tensor("kq", [NLEV, CQ], f32, kind="ExternalInput").ap()
    ka_d = nc.dram_tensor("ka", [NLEV, P, NB], f32, kind="ExternalInput").ap()
    madd_d = nc.dram_tensor("madd", [NVS, NLEV, 1], f32, kind="ExternalInput").ap()
    stats_d = nc.dram_tensor(
        "stats", [NVS, NLEV, 3, P, NB], f32, kind="ExternalOutput").ap()

    with tile.TileContext(nc) as tc, ExitStack() as ctx:
        const_pool = ctx.enter_context(tc.tile_pool(name="const", bufs=1))
        fqt_pool = ctx.enter_context(tc.tile_pool(name="fqt", bufs=2))
        kq_pool = ctx.enter_context(tc.tile_pool(name="kqb", bufs=2))
        scr_pool = ctx.enter_context(tc.tile_pool(name="scr", bufs=4))
        st_pool = ctx.enter_context(tc.tile_pool(name="st", bufs=2))
        psum_pool = ctx.enter_context(tc.tile_pool(name="ps", bufs=3, space="PSUM"))

        if two_pass:
            ft_hi = const_pool.tile([P, NK, B], bf16)
            nc.sync.dma_start(out=ft_hi, in_=ft_hi_d.rearrange("(k p) b -> p k b", p=P))
            ft_lo = const_pool.tile([P, NK, B], bf16)
            nc.sync.dma_start(out=ft_lo, in_=ft_lo_d.rearrange("(k p) b -> p k b", p=P))
        else:
            ft_sb = const_pool.tile([P, NK, B], mmdt)
            nc.sync.dma_start(out=ft_sb, in_=ft_d.rearrange("(k p) b -> p k b", p=P))
        ka_sb = const_pool.tile([P, NLEV, NB], f32)
        nc.sync.dma_start(out=ka_sb, in_=ka_d.rearrange("l p c -> p l c"))

        for v in range(NVS):
            if two_pass:
                fqt_hi = fqt_pool.tile([P, NK, QS], bf16, tag="fqt_hi")
                nc.sync.dma_start(
                    out=fqt_hi,
                    in_=fqt_hi_d[:, v * QS:(v + 1) * QS].rearrange("(k p) q -> p k q", p=P))
                fqt_lo = fqt_pool.tile([P, NK, QS], bf16, tag="fqt_lo")
                nc.sync.dma_start(
                    out=fqt_lo,
                    in_=fqt_lo_d[:, v * QS:(v + 1) * QS].rearrange("(k p) q -> p k q", p=P))
            else:
                fqt_sb = fqt_pool.tile([P, NK, QS], mmdt)
                nc.sync.dma_start(
                    out=fqt_sb,
                    in_=fqt_d[:, v * QS:(v + 1) * QS].rearrange("(k p) q -> p k q", p=P))

            kqb = []
            for li in range(NLEV):
                n = nmx[li]
                if n == 0:
                    kqb.append(None)
                    continue
                t = kq_pool.tile([P, nmx[0]], f32, tag=f"kqb{li}")
                nc.gpsimd.dma_start(
                    out=t[:, :n],
                    in_=kq_d[li:li + 1, v * QS: v * QS + n].to_broadcast([P, n]))
                kqb.append(t)
            mt = {}
            for li in range(1, NLEV):
                w = nmx[li] - nmn[li]
                if nmx[li] > 0 and w > 0:
                    t = kq_pool.tile([P, w], f32, tag=f"madd{li}")
                    nc.gpsimd.dma_start(
                        out=t, in_=madd_d[v, li:li + 1, 0].to_broadcast([P, w]))
                    mt[li] = t

            neglm_t = [st_pool.tile([P, NB], f32, tag=f"nl{li}", name=f"nl{li}_{v}")
                       for li in range(NLEV)]
            den_t = [st_pool.tile([P, NB], f32, tag=f"dn{li}", name=f"dn{li}_{v}")
                     for li in range(NLEV)]
            pos_t = [st_pool.tile([P, NB], f32, tag=f"po{li}", name=f"po{li}_{v}")
                     for li in range(NLEV)]

            for c in range(NB):
                ps = psum_pool.tile([P, QS], f32)
                for g in range(QS // 512):
                    gs = slice(g * 512, (g + 1) * 512)
                    if two_pass:
                        passes = [(ft_hi, fqt_hi), (ft_hi, fqt_lo), (ft_lo, fqt_hi)]
                        n_mm = len(passes) * NK
                        i = 0
                        for lt, rt in passes:
                            for k in range(NK):
                                nc.tensor.matmul(
                                    ps[:, gs],
                                    lt[:, k, c * P:(c + 1) * P],
                                    rt[:, k, gs],
                                    start=(i == 0), stop=(i == n_mm - 1))
                                i += 1
                    else:
                        for k in range(NK):
                            nc.tensor.matmul(
                                ps[:, gs],
                                ft_sb[:, k, c * P:(c + 1) * P],
                                fqt_sb[:, k, gs],
                                start=(k == 0), stop=(k == NK - 1))

                for li in range(3):
                    n = nmx[li]
                    if n == 0:
                        continue
                    if li in mt:
                        nc.vector.tensor_add(
                            ps[:, nmn[li]:nmx[li]], ps[:, nmn[li]:nmx[li]], mt[li])
                    nc.vector.tensor_reduce(
                        neglm_t[li][:, c:c + 1], ps[:, :n],
                        axis=mybir.AxisListType.X, op=mybir.AluOpType.max,
                        negate=True)
                    e_scr = scr_pool.tile([P, nmx[0]], f32, tag="escr")
                    nc.scalar.activation(
                        e_scr[:, :n], ps[:, :n],
                        mybir.ActivationFunctionType.Exp,
                        bias=neglm_t[li][:, c:c + 1], scale=1.0,
                        accum_out=den_t[li][:, c:c + 1])
                    m_scr = scr_pool.tile([P, nmx[0]], f32, tag="mscr")
                    nc.vector.scalar_tensor_tensor(
                        out=m_scr[:, :n], in0=kqb[li][:, :n],
                        scalar=ka_sb[:, li, c:c + 1], in1=ps[:, :n],
                        op0=mybir.AluOpType.is_equal, op1=mybir.AluOpType.mult,
                        accum_out=pos_t[li][:, c:c + 1])

            for li in range(NLEV):
                for si, t in ((0, neglm_t[li]), (1, den_t[li]), (2, pos_t[li])):
                    nc.sync.dma_start(out=stats_d[v, li, si], in_=t)

    nc.compile()
    return nc


# ----------------------------------------------------------------- host prep
def _prepare(features, labels, features_queue, labels_queue):
    """Host-side: masks, balanced shard assignment, per-core input arrays."""
    B, D = features.shape
    Q = features_queue.shape[0]
    S = NCORES * NVS
    QS_SHARD = Q // S
    NB = B // P
    NLEV = 3

    levels = _host_masks(labels, labels_queue)

    # lifetime = last level at which a queue column is active (1..3)
    life = np.ones(Q, np.int64)
    for li in (1, 2):
        life += levels[li]["queue_active"].astype(np.int64)
    order_cols = np.argsort(-life, kind="stable")
    perm = order_cols.reshape(QS_SHARD, S).T  # [S, QS_SHARD]: shard s -> cols

    n_per_shard = np.zeros((S, NLEV), np.int64)
    n_per_shard[:, 0] = QS_SHARD
    for li in (1, 2):
        n_per_shard[:, li] = levels[li]["queue_active"][perm].sum(axis=1)
    nmx = [int(n_per_shard[:, li].max()) for li in range(NLEV)]
    nmn = [int(n_per_shard[:, li].min()) for li in range(NLEV)]
    assert nmx[0] == nmn[0] == QS_SHARD
    for li in range(1, NLEV):
        assert nmx[li] - nmn[li] <= 1, (nmx, nmn)

    # ---- per-core input arrays
    ftS = np.ascontiguousarray((features / TEMP).T)  # [D, B]
    fqT = np.ascontiguousarray(features_queue.T)     # [D, Q]

    ka_r = np.empty((NLEV, P, NB), np.float32)
    for li in range(NLEV):
        ka_r[li] = levels[li]["ka"].astype(np.float32).reshape(NB, P).T

    in_maps = []
    for c in range(NCORES):
        cols = perm[c * NVS:(c + 1) * NVS].reshape(-1)  # [CQ]
        fqt_c = np.ascontiguousarray(fqT[:, cols])
        kq_c = np.empty((NLEV, NVS * QS_SHARD), np.float32)
        for li in range(NLEV):
            kq_c[li] = np.where(
                levels[li]["queue_active"][cols],
                levels[li]["kq"][cols].astype(np.float32), np.float32(-1.0))
        madd_c = np.zeros((NVS, NLEV, 1), np.float32)
        for v in range(NVS):
            s = c * NVS + v
            for li in range(1, NLEV):
                if nmx[li] - nmn[li] > 0:
                    # mask the single boundary column if dead for this shard
                    madd_c[v, li, 0] = (
                        np.float32(MASK_VAL)
                        if n_per_shard[s, li] < nmx[li] else np.float32(0.0))
        m = {"kq": kq_c, "ka": ka_r, "madd": madd_c}
        if MM_MODE == "bf16x3":
            import ml_dtypes
            bf = ml_dtypes.bfloat16
            ft_hi = ftS.astype(bf)
            ft_lo = (ftS - ft_hi.astype(np.float32)).astype(bf)
            fq_hi = fqt_c.astype(bf)
            fq_lo = (fqt_c - fq_hi.astype(np.float32)).astype(bf)
            m.update(ft_hi=ft_hi, ft_lo=ft_lo, fqt_hi=fq_hi, fqt_lo=fq_lo)
        else:
            m.update(ft=ftS, fqt=fqt_c)
        in_maps.append(m)

    return dict(in_maps=in_maps, levels=levels, perm=perm,
                n_per_shard=n_per_shard, nmx=nmx, nmn=nmn,
                B=B, D=D, Q=Q, S=S, QS_SHARD=QS_SHARD, NB=NB, NLEV=NLEV)


# -------------------------------------------------------------------- kernel
def kernel(features, labels, features_queue, labels_queue):
    t0 = time.time()
    features = np.asarray(features, dtype=np.float32)
    features_queue = np.asarray(features_queue, dtype=np.float32)
    labels = np.asarray(labels)
    labels_queue = np.asarray(labels_queue)

    prep = _prepare(features, labels, features_queue, labels_queue)
    in_maps = prep["in_maps"]
    levels = prep["levels"]
    n_per_shard = prep["n_per_shard"]
    nmx, nmn = prep["nmx"], prep["nmn"]
    B, D = prep["B"], prep["D"]
    S, QS_SHARD = prep["S"], prep["QS_SHARD"]
    NLEV = prep["NLEV"]
    t_prep = time.time() - t0

    # ---- build + run device program
    t0 = time.time()
    nc = _build_program(D, B, NVS * QS_SHARD, nmx, nmn, MM_MODE)
    t_build = time.time() - t0

    t0 = time.time()
    br = run_bass_kernel_spmd(nc, in_maps, core_ids=list(range(NCORES)))
    t_run = time.time() - t0

    LAST_RUN.clear()
    LAST_RUN.update(
        exec_time_ns=br.exec_time_ns,
        mean_exec_time_ns=getattr(br, "mean_exec_time_ns", None),
        t_prep=t_prep, t_build=t_build, t_run=t_run,
        profile_json=br.profile_json,
        instructions_and_trace=br.instructions_and_trace,
        nmx=nmx, nmn=nmn)

    # ---- host merge (float64)
    t0 = time.time()
    # stats[c]: [NVS, NLEV, 3, P, NB] -> per shard arrays [B]
    neg_lm = np.empty((S, NLEV, B), np.float64)
    den = np.empty((S, NLEV, B), np.float64)
    pos = np.empty((S, NLEV, B), np.float64)
    for c in range(NCORES):
        st = br.results[c]["stats"]  # [NVS, NLEV, 3, P, NB]
        for v in range(NVS):
            s = c * NVS + v
            for li in range(NLEV):
                neg_lm[s, li] = st[v, li, 0].T.reshape(-1)
                den[s, li] = st[v, li, 1].T.reshape(-1)
                pos[s, li] = st[v, li, 2].T.reshape(-1)

    cum = 0.0
    max_lower = -np.inf
    for li in range(NLEV):
        l = li + 1
        cnt = levels[li]["cnt"].astype(np.float64)
        valid = n_per_shard[:, li] > 0  # shards with any columns at this level
        lm_s = -neg_lm[valid, li]      # [S', B]
        den_s = den[valid, li]
        pos_s = pos[valid, li]
        if lm_s.shape[0] == 0:
            layer_loss = 0.0
        else:
            lm = lm_s.max(axis=0)
            dtot = (den_s * np.exp(lm_s - lm[None, :])).sum(axis=0)
            ptot = pos_s.sum(axis=0)
            with np.errstate(divide="ignore", invalid="ignore"):
                mean = (ptot - cnt * (lm + np.log(dtot))) / (cnt + 1e-12)
            mean = np.where(cnt > 0, mean, 0.0)
            loss_i = -(TEMP / BASE_TEMP) * mean
            num = float((cnt > 0).sum())
            layer_loss = float(loss_i.sum() / (num + 1e-12))
        layer_loss = max(max_lower, layer_loss)
        cum = cum + (2.0 ** (1.0 / l)) * layer_loss
        max_lower = max(max_lower, layer_loss)

    LAST_RUN["t_merge"] = time.time() - t0
    return np.float32(cum)


# revision 12
# speedup vs baseline: 1.4697x; 1.0129x over previous
"""HMLC loss kernel for 8 Trainium2 NeuronCores (Bass/Tile).

Strategy (queue-sharded data parallelism):
  * All mask/dedup/queue-evolution logic in the reference depends ONLY on the
    integer labels -> computed exactly on host (numpy).
  * The queue (32768 cols) is split into 32 shards (8 cores x 4 vshards).
    Within each shard, columns are ordered by "lifetime" (the last level at
    which the column is still active), so the active set at every level is a
    prefix. The assignment is round-robin over the lifetime-sorted global
    column order, so prefix lengths differ by at most 1 across shards and a
    single compiled SPMD program (prefix bounds baked as max over shards)
    serves all cores; a <=1-column-wide additive -6e4 mask (per-core DATA)
    handles the remainder.
  * Device per (vshard, anchor-chunk): PE computes sim = (f/TEMP) @ fq_shard.T
    into PSUM [128,1024]; per level the stats are
        neg_lm  = -max(sim[:, :n])                  (VectorE tensor_reduce)
        denom   = sum exp(sim[:, :n] + neg_lm)      (ScalarE activation+accum)
        possum  = sum (kq==ka) * sim[:, :n]         (VectorE scalar_tensor_tensor+accum)
  * Host merges the 32 shards per level (online softmax) in float64 and runs
    the scalar hmce chain.
"""

import os
import sys
import time
from contextlib import ExitStack

if "/opt/trn_rl_repo" not in sys.path:
    sys.path.insert(0, "/opt/trn_rl_repo")

import numpy as np

import concourse.bass as bass  # noqa: E402
import concourse.bacc as bacc  # noqa: E402
import concourse.tile as tile  # noqa: E402
from concourse import mybir  # noqa: E402
from concourse.bass_utils import run_bass_kernel_spmd  # noqa: E402

TEMP = 0.07
BASE_TEMP = 0.07
NCORES = 8
NVS = 4          # vshards per core
P = 128          # partitions
MASK_VAL = -60000.0

# matmul precision mode: "f32" (exact, 4 cyc/row), "f32r" (1 cyc/row),
# "bf16x3" (hi/lo split, 3 passes, 1 cyc/row each)
MM_MODE = os.environ.get("HMLC_MM_MODE", "f32")

# populated by kernel() for test harness introspection
LAST_RUN = {}


# ---------------------------------------------------------------- host masks
def _host_masks(labels, labels_queue):
    """Exact replication of the reference's label-only mask evolution."""
    B, L = labels.shape
    Q = labels_queue.shape[0]
    base = int(max(labels.max(), labels_queue.max())) + 1
    pw = base ** np.arange(L - 1, -1, -1)

    anchor_active = np.ones(B, bool)
    queue_active = np.ones(Q, bool)
    order = np.arange(B)

    levels = []
    for l in range(1, L):
        ncols = L - l
        w = (pw * (np.arange(L) < ncols)).astype(np.int64)
        ka = labels.astype(np.int64) @ w
        kq = labels_queue.astype(np.int64) @ w
        maxk = int(max(ka.max(), kq.max())) + 1
        bc = np.bincount(kq[queue_active], minlength=maxk)
        cnt = np.where(anchor_active, bc[ka], 0)
        pres = np.zeros(maxk, bool)
        pres[ka[anchor_active]] = True
        newmatch = queue_active & pres[kq]
        levels.append(dict(
            ka=ka.copy(), kq=kq.copy(),
            queue_active=queue_active.copy(),
            cnt=cnt.copy(),
        ))
        same = (ka[:, None] == ka[None, :]) & anchor_active[:, None] & anchor_active[None, :]
        max_ord = np.max(np.where(same, order[None, :], -1), axis=1)
        kept = anchor_active & (order == max_ord)
        rank = (kept[None, :] & (ka[None, :] < ka[:, None])).sum(1)
        order = np.where(kept, rank, -1)
        anchor_active = kept
        queue_active = queue_active & ~newmatch
    return levels


# ------------------------------------------------------------ device program
def _build_program(D, B, CQ, nmx, nmn, mm_mode):
    NLEV = 3
    f32 = mybir.dt.float32
    NB = B // P       # anchor chunks
    NK = D // P       # contraction chunks
    QS = CQ // NVS    # vshard width

    nc = bacc.Bacc("TRN2", target_bir_lowering=False, debug=False)

    two_pass = mm_mode == "bf16x3"
    if two_pass:
        bf16 = mybir.dt.bfloat16
        ft_hi_d = nc.dram_tensor("ft_hi", [D, B], bf16, kind="ExternalInput").ap()
        ft_lo_d = nc.dram_tensor("ft_lo", [D, B], bf16, kind="ExternalInput").ap()
        fqt_hi_d = nc.dram_tensor("fqt_hi", [D, CQ], bf16, kind="ExternalInput").ap()
        fqt_lo_d = nc.dram_tensor("fqt_lo", [D, CQ], bf16, kind="ExternalInput").ap()
    else:
        mmdt = mybir.dt.float32r if mm_mode == "f32r" else f32
        ft_d = nc.dram_tensor("ft", [D, B], mmdt, kind="ExternalInput").ap()
        fqt_d = nc.dram_tensor("fqt", [D, CQ], mmdt, kind="ExternalInput").ap()
    kq_d = nc.dram_tensor("kq", [NLEV, CQ], f32, kind="ExternalInput").ap()
    ka_d = nc.dram_tensor("ka", [NLEV, P, NB], f32, kind="ExternalInput").ap()
    madd_d = nc.dram_tensor("madd", [NVS, NLEV, 1], f32, kind="ExternalInput").ap()
    stats_d = nc.dram_tensor(
        "stats", [NVS, NLEV, 3, P, NB], f32, kind="ExternalOutput").ap()

    with tile.TileContext(nc) as tc, ExitStack() as ctx:
        const_pool = ctx.enter_context(tc.tile_pool(name="const", bufs=1))
        fqt_pool = ctx.enter_context(tc.tile_pool(name="fqt", bufs=2))
        kq_pool = ctx.enter_context(tc.tile_pool(name="kqb", bufs=2))
        scr_pool = ctx.enter_context(tc.tile_pool(name="scr", bufs=4))
        st_pool = ctx.enter_context(tc.tile_pool(name="st", bufs=2))
        psum_pool = ctx.enter_context(tc.tile_pool(name="ps", bufs=4, space="PSUM"))

        if two_pass:
            ft_hi = const_pool.tile([P, NK, B], bf16)
            nc.sync.dma_start(out=ft_hi, in_=ft_hi_d.rearrange("(k p) b -> p k b", p=P))
            ft_lo = const_pool.tile([P, NK, B], bf16)
            nc.sync.dma_start(out=ft_lo, in_=ft_lo_d.rearrange("(k p) b -> p k b", p=P))
        else:
            ft_sb = const_pool.tile([P, NK, B], mmdt)
            nc.sync.dma_start(out=ft_sb, in_=ft_d.rearrange("(k p) b -> p k b", p=P))
        ka_sb = const_pool.tile([P, NLEV, NB], f32)
        nc.sync.dma_start(out=ka_sb, in_=ka_d.rearrange("l p c -> p l c"))

        for v in range(NVS):
            if two_pass:
                fqt_hi = fqt_pool.tile([P, NK, QS], bf16, tag="fqt_hi")
                nc.sync.dma_start(
                    out=fqt_hi,
                    in_=fqt_hi_d[:, v * QS:(v + 1) * QS].rearrange("(k p) q -> p k q", p=P))
                fqt_lo = fqt_pool.tile([P, NK, QS], bf16, tag="fqt_lo")
                nc.sync.dma_start(
                    out=fqt_lo,
                    in_=fqt_lo_d[:, v * QS:(v + 1) * QS].rearrange("(k p) q -> p k q", p=P))
            else:
                fqt_sb = fqt_pool.tile([P, NK, QS], mmdt)
                nc.sync.dma_start(
                    out=fqt_sb,
                    in_=fqt_d[:, v * QS:(v + 1) * QS].rearrange("(k p) q -> p k q", p=P))

            kqb = []
            for li in range(NLEV):
                n = nmx[li]
                if n == 0:
                    kqb.append(None)
                    continue
                t = kq_pool.tile([P, nmx[0]], f32, tag=f"kqb{li}")
                nc.gpsimd.dma_start(
                    out=t[:, :n],
                    in_=kq_d[li:li + 1, v * QS: v * QS + n].to_broadcast([P, n]))
                kqb.append(t)
            mt = {}
            for li in range(1, NLEV):
                w = nmx[li] - nmn[li]
                if nmx[li] > 0 and w > 0:
                    t = kq_pool.tile([P, w], f32, tag=f"madd{li}")
                    nc.gpsimd.dma_start(
                        out=t, in_=madd_d[v, li:li + 1, 0].to_broadcast([P, w]))
                    mt[li] = t

            neglm_t = [st_pool.tile([P, NB], f32, tag=f"nl{li}", name=f"nl{li}_{v}")
                       for li in range(NLEV)]
            den_t = [st_pool.tile([P, NB], f32, tag=f"dn{li}", name=f"dn{li}_{v}")
                     for li in range(NLEV)]
            pos_t = [st_pool.tile([P, NB], f32, tag=f"po{li}", name=f"po{li}_{v}")
                     for li in range(NLEV)]

            for c in range(NB):
                ps = psum_pool.tile([P, QS], f32)
                ngr = QS // 512
                if two_pass:
                    passes = [(ft_hi, fqt_hi), (ft_hi, fqt_lo), (ft_lo, fqt_hi)]
                    for pi, (lt, rt) in enumerate(passes):
                        for k in range(NK):
                            for g in range(ngr):
                                gs = slice(g * 512, (g + 1) * 512)
                                nc.tensor.matmul(
                                    ps[:, gs],
                                    lt[:, k, c * P:(c + 1) * P],
                                    rt[:, k, gs],
                                    start=(pi == 0 and k == 0),
                                    stop=(pi == len(passes) - 1 and k == NK - 1))
                else:
                    # k outer / group inner: both 512-wide groups reuse the
                    # same stationary weight load
                    for k in range(NK):
                        for g in range(ngr):
                            gs = slice(g * 512, (g + 1) * 512)
                            nc.tensor.matmul(
                                ps[:, gs],
                                ft_sb[:, k, c * P:(c + 1) * P],
                                fqt_sb[:, k, gs],
                                start=(k == 0), stop=(k == NK - 1))

                for li in range(3):
                    n = nmx[li]
                    if n == 0:
                        continue
                    if li in mt:
                        nc.vector.tensor_add(
                            ps[:, nmn[li]:nmx[li]], ps[:, nmn[li]:nmx[li]], mt[li])
                    nc.vector.tensor_reduce(
                        neglm_t[li][:, c:c + 1], ps[:, :n],
                        axis=mybir.AxisListType.X, op=mybir.AluOpType.max,
                        negate=True)
                    e_scr = scr_pool.tile([P, nmx[0]], f32, tag="escr")
                    nc.scalar.activation(
                        e_scr[:, :n], ps[:, :n],
                        mybir.ActivationFunctionType.Exp,
                        bias=neglm_t[li][:, c:c + 1], scale=1.0,
                        accum_out=den_t[li][:, c:c + 1])
                    m_scr = scr_pool.tile([P, nmx[0]], f32, tag="mscr")
                    nc.vector.scalar_tensor_tensor(
                        out=m_scr[:, :n], in0=kqb[li][:, :n],
                        scalar=ka_sb[:, li, c:c + 1], in1=ps[:, :n],
                        op0=mybir.AluOpType.is_equal, op1=mybir.AluOpType.mult,
                        accum_out=pos_t[li][:, c:c + 1])

            for li in range(NLEV):
                for si, t in ((0, neglm_t[li]), (1, den_t[li]), (2, pos_t[li])):
                    nc.sync.dma_start(out=stats_d[v, li, si], in_=t)

    nc.compile()
    return nc


# ----------------------------------------------------------------- host prep
def _prepare(features, labels, features_queue, labels_queue):
    """Host-side: masks, balanced shard assignment, per-core input arrays."""
    B, D = features.shape
    Q = features_queue.shape[0]
    S = NCORES * NVS
    QS_SHARD = Q // S
    NB = B // P
    NLEV = 3

    levels = _host_masks(labels, labels_queue)

    # lifetime = last level at which a queue column is active (1..3)
    life = np.ones(Q, np.int64)
    for li in (1, 2):
        life += levels[li]["queue_active"].astype(np.int64)
    order_cols = np.argsort(-life, kind="stable")
    perm = order_cols.reshape(QS_SHARD, S).T  # [S, QS_SHARD]: shard s -> cols

    n_per_shard = np.zeros((S, NLEV), np.int64)
    n_per_shard[:, 0] = QS_SHARD
    for li in (1, 2):
        n_per_shard[:, li] = levels[li]["queue_active"][perm].sum(axis=1)
    nmx = [int(n_per_shard[:, li].max()) for li in range(NLEV)]
    nmn = [int(n_per_shard[:, li].min()) for li in range(NLEV)]
    assert nmx[0] == nmn[0] == QS_SHARD
    for li in range(1, NLEV):
        assert nmx[li] - nmn[li] <= 1, (nmx, nmn)

    # ---- per-core input arrays
    ftS = np.ascontiguousarray((features / TEMP).T)  # [D, B]
    fqT = np.ascontiguousarray(features_queue.T)     # [D, Q]

    ka_r = np.empty((NLEV, P, NB), np.float32)
    for li in range(NLEV):
        ka_r[li] = levels[li]["ka"].astype(np.float32).reshape(NB, P).T

    in_maps = []
    for c in range(NCORES):
        cols = perm[c * NVS:(c + 1) * NVS].reshape(-1)  # [CQ]
        fqt_c = np.ascontiguousarray(fqT[:, cols])
        kq_c = np.empty((NLEV, NVS * QS_SHARD), np.float32)
        for li in range(NLEV):
            kq_c[li] = np.where(
                levels[li]["queue_active"][cols],
                levels[li]["kq"][cols].astype(np.float32), np.float32(-1.0))
        madd_c = np.zeros((NVS, NLEV, 1), np.float32)
        for v in range(NVS):
            s = c * NVS + v
            for li in range(1, NLEV):
                if nmx[li] - nmn[li] > 0:
                    # mask the single boundary column if dead for this shard
                    madd_c[v, li, 0] = (
                        np.float32(MASK_VAL)
                        if n_per_shard[s, li] < nmx[li] else np.float32(0.0))
        m = {"kq": kq_c, "ka": ka_r, "madd": madd_c}
        if MM_MODE == "bf16x3":
            import ml_dtypes
            bf = ml_dtypes.bfloat16
            ft_hi = ftS.astype(bf)
            ft_lo = (ftS - ft_hi.astype(np.float32)).astype(bf)
            fq_hi = fqt_c.astype(bf)
            fq_lo = (fqt_c - fq_hi.astype(np.float32)).astype(bf)
            m.update(ft_hi=ft_hi, ft_lo=ft_lo, fqt_hi=fq_hi, fqt_lo=fq_lo)
        else:
            m.update(ft=ftS, fqt=fqt_c)
        in_maps.append(m)

    return dict(in_maps=in_maps, levels=levels, perm=perm,
                n_per_shard=n_per_shard, nmx=nmx, nmn=nmn,
                B=B, D=D, Q=Q, S=S, QS_SHARD=QS_SHARD, NB=NB, NLEV=NLEV)


# -------------------------------------------------------------------- kernel
def kernel(features, labels, features_queue, labels_queue):
    t0 = time.time()
    features = np.asarray(features, dtype=np.float32)
    features_queue = np.asarray(features_queue, dtype=np.float32)
    labels = np.asarray(labels)
    labels_queue = np.asarray(labels_queue)

    prep = _prepare(features, labels, features_queue, labels_queue)
    in_maps = prep["in_maps"]
    levels = prep["levels"]
    n_per_shard = prep["n_per_shard"]
    nmx, nmn = prep["nmx"], prep["nmn"]
    B, D = prep["B"], prep["D"]
    S, QS_SHARD = prep["S"], prep["QS_SHARD"]
    NLEV = prep["NLEV"]
    t_prep = time.time() - t0

    # ---- build + run device program
    t0 = time.time()
    nc = _build_program(D, B, NVS * QS_SHARD, nmx, nmn, MM_MODE)
    t_build = time.time() - t0

    t0 = time.time()
    br = run_bass_kernel_spmd(nc, in_maps, core_ids=list(range(NCORES)))
    t_run = time.time() - t0

    LAST_RUN.clear()
    LAST_RUN.update(
        exec_time_ns=br.exec_time_ns,
        mean_exec_time_ns=getattr(br, "mean_exec_time_ns", None),
        t_prep=t_prep, t_build=t_build, t_run=t_run,
        profile_json=br.profile_json,
        instructions_and_trace=br.instructions_and_trace,
        nmx=nmx, nmn=nmn)

    # ---- host merge (float64)
    t0 = time.time()
    # stats[c]: [NVS, NLEV, 3, P, NB] -> per shard arrays [B]
    neg_lm = np.empty((S, NLEV, B), np.float64)
    den = np.empty((S, NLEV, B), np.float64)
    pos = np.empty((S, NLEV, B), np.float64)
    for c in range(NCORES):
        st = br.results[c]["stats"]  # [NVS, NLEV, 3, P, NB]
        for v in range(NVS):
            s = c * NVS + v
            for li in range(NLEV):
                neg_lm[s, li] = st[v, li, 0].T.reshape(-1)
                den[s, li] = st[v, li, 1].T.reshape(-1)
                pos[s, li] = st[v, li, 2].T.reshape(-1)

    cum = 0.0
    max_lower = -np.inf
    for li in range(NLEV):
        l = li + 1
        cnt = levels[li]["cnt"].astype(np.float64)
        valid = n_per_shard[:, li] > 0  # shards with any columns at this level
        lm_s = -neg_lm[valid, li]      # [S', B]
        den_s = den[valid, li]
        pos_s = pos[valid, li]
        if lm_s.shape[0] == 0:
            layer_loss = 0.0
        else:
            lm = lm_s.max(axis=0)
            dtot = (den_s * np.exp(lm_s - lm[None, :])).sum(axis=0)
            ptot = pos_s.sum(axis=0)
            with np.errstate(divide="ignore", invalid="ignore"):
                mean = (ptot - cnt * (lm + np.log(dtot))) / (cnt + 1e-12)
            mean = np.where(cnt > 0, mean, 0.0)
            loss_i = -(TEMP / BASE_TEMP) * mean
            num = float((cnt > 0).sum())
            layer_loss = float(loss_i.sum() / (num + 1e-12))
        layer_loss = max(max_lower, layer_loss)
        cum = cum + (2.0 ** (1.0 / l)) * layer_loss
        max_lower = max(max_lower, layer_loss)

    LAST_RUN["t_merge"] = time.time() - t0
    return np.float32(cum)


# revision 14
# speedup vs baseline: 1.4929x; 1.0158x over previous
"""HMLC loss kernel for 8 Trainium2 NeuronCores (Bass/Tile).

Strategy (queue-sharded data parallelism):
  * All mask/dedup/queue-evolution logic in the reference depends ONLY on the
    integer labels -> computed exactly on host (numpy).
  * The queue (32768 cols) is split into 32 shards (8 cores x 4 vshards).
    Within each shard, columns are ordered by "lifetime" (the last level at
    which the column is still active), so the active set at every level is a
    prefix. The assignment is round-robin over the lifetime-sorted global
    column order, so prefix lengths differ by at most 1 across shards and a
    single compiled SPMD program (prefix bounds baked as max over shards)
    serves all cores; a <=1-column-wide additive -6e4 mask (per-core DATA)
    handles the remainder.
  * Device per (vshard, anchor-chunk): PE computes sim = (f/TEMP) @ fq_shard.T
    into PSUM [128,1024]; per level the stats are
        neg_lm  = -max(sim[:, :n])                  (VectorE tensor_reduce)
        denom   = sum exp(sim[:, :n] + neg_lm)      (ScalarE activation+accum)
        possum  = sum (kq==ka) * sim[:, :n]         (VectorE scalar_tensor_tensor+accum)
  * Host merges the 32 shards per level (online softmax) in float64 and runs
    the scalar hmce chain.
"""

import os
import sys
import time
from contextlib import ExitStack

if "/opt/trn_rl_repo" not in sys.path:
    sys.path.insert(0, "/opt/trn_rl_repo")

import numpy as np

import concourse.bass as bass  # noqa: E402
import concourse.bacc as bacc  # noqa: E402
import concourse.tile as tile  # noqa: E402
from concourse import mybir  # noqa: E402
from concourse.bass_utils import run_bass_kernel_spmd  # noqa: E402

TEMP = 0.07
BASE_TEMP = 0.07
NCORES = 8
NVS = 4          # vshards per core
P = 128          # partitions
MASK_VAL = -60000.0

# matmul precision mode: "f32" (exact, 4 cyc/row), "f32r" (1 cyc/row),
# "bf16x3" (hi/lo split, 3 passes, 1 cyc/row each)
MM_MODE = os.environ.get("HMLC_MM_MODE", "f32")

# populated by kernel() for test harness introspection
LAST_RUN = {}


# ---------------------------------------------------------------- host masks
def _host_masks(labels, labels_queue):
    """Exact replication of the reference's label-only mask evolution."""
    B, L = labels.shape
    Q = labels_queue.shape[0]
    base = int(max(labels.max(), labels_queue.max())) + 1
    pw = base ** np.arange(L - 1, -1, -1)

    anchor_active = np.ones(B, bool)
    queue_active = np.ones(Q, bool)
    order = np.arange(B)

    levels = []
    for l in range(1, L):
        ncols = L - l
        w = (pw * (np.arange(L) < ncols)).astype(np.int64)
        ka = labels.astype(np.int64) @ w
        kq = labels_queue.astype(np.int64) @ w
        maxk = int(max(ka.max(), kq.max())) + 1
        bc = np.bincount(kq[queue_active], minlength=maxk)
        cnt = np.where(anchor_active, bc[ka], 0)
        pres = np.zeros(maxk, bool)
        pres[ka[anchor_active]] = True
        newmatch = queue_active & pres[kq]
        levels.append(dict(
            ka=ka.copy(), kq=kq.copy(),
            queue_active=queue_active.copy(),
            cnt=cnt.copy(),
        ))
        same = (ka[:, None] == ka[None, :]) & anchor_active[:, None] & anchor_active[None, :]
        max_ord = np.max(np.where(same, order[None, :], -1), axis=1)
        kept = anchor_active & (order == max_ord)
        rank = (kept[None, :] & (ka[None, :] < ka[:, None])).sum(1)
        order = np.where(kept, rank, -1)
        anchor_active = kept
        queue_active = queue_active & ~newmatch
    return levels


# ------------------------------------------------------------ device program
def _build_program(D, B, CQ, nmx, nmn, mm_mode):
    NLEV = 3
    f32 = mybir.dt.float32
    NB = B // P       # anchor chunks
    NK = D // P       # contraction chunks
    QS = CQ // NVS    # vshard width

    nc = bacc.Bacc("TRN2", target_bir_lowering=False, debug=False)

    two_pass = mm_mode == "bf16x3"
    if two_pass:
        bf16 = mybir.dt.bfloat16
        ft_hi_d = nc.dram_tensor("ft_hi", [D, B], bf16, kind="ExternalInput").ap()
        ft_lo_d = nc.dram_tensor("ft_lo", [D, B], bf16, kind="ExternalInput").ap()
        fqt_hi_d = nc.dram_tensor("fqt_hi", [D, CQ], bf16, kind="ExternalInput").ap()
        fqt_lo_d = nc.dram_tensor("fqt_lo", [D, CQ], bf16, kind="ExternalInput").ap()
    else:
        mmdt = mybir.dt.float32r if mm_mode == "f32r" else f32
        ft_d = nc.dram_tensor("ft", [D, B], mmdt, kind="ExternalInput").ap()
        fqt_d = nc.dram_tensor("fqt", [D, CQ], mmdt, kind="ExternalInput").ap()
    kq_d = nc.dram_tensor("kq", [NLEV, CQ], f32, kind="ExternalInput").ap()
    ka_d = nc.dram_tensor("ka", [NLEV, P, NB], f32, kind="ExternalInput").ap()
    madd_d = nc.dram_tensor("madd", [NVS, NLEV, 1], f32, kind="ExternalInput").ap()
    stats_d = nc.dram_tensor(
        "stats", [NVS, NLEV, 3, P, NB], f32, kind="ExternalOutput").ap()

    with tile.TileContext(nc) as tc, ExitStack() as ctx:
        const_pool = ctx.enter_context(tc.tile_pool(name="const", bufs=1))
        fqt_pool = ctx.enter_context(tc.tile_pool(name="fqt", bufs=2))
        kq_pool = ctx.enter_context(tc.tile_pool(name="kqb", bufs=2))
        scr_pool = ctx.enter_context(tc.tile_pool(name="scr", bufs=4))
        st_pool = ctx.enter_context(tc.tile_pool(name="st", bufs=2))
        psum_pool = ctx.enter_context(tc.tile_pool(name="ps", bufs=4, space="PSUM"))

        if two_pass:
            ft_hi = const_pool.tile([P, NK, B], bf16)
            nc.sync.dma_start(out=ft_hi, in_=ft_hi_d.rearrange("(k p) b -> p k b", p=P))
            ft_lo = const_pool.tile([P, NK, B], bf16)
            nc.sync.dma_start(out=ft_lo, in_=ft_lo_d.rearrange("(k p) b -> p k b", p=P))
        else:
            ft_sb = const_pool.tile([P, NK, B], mmdt)
            ft_r = ft_d.rearrange("(k p) b -> p k b", p=P)
            for k in range(NK):
                nc.sync.dma_start(out=ft_sb[:, k, :], in_=ft_r[:, k, :])
        ka_sb = const_pool.tile([P, NLEV, NB], f32)
        nc.sync.dma_start(out=ka_sb, in_=ka_d.rearrange("l p c -> p l c"))

        for v in range(NVS):
            if two_pass:
                fqt_hi = fqt_pool.tile([P, NK, QS], bf16, tag="fqt_hi")
                nc.sync.dma_start(
                    out=fqt_hi,
                    in_=fqt_hi_d[:, v * QS:(v + 1) * QS].rearrange("(k p) q -> p k q", p=P))
                fqt_lo = fqt_pool.tile([P, NK, QS], bf16, tag="fqt_lo")
                nc.sync.dma_start(
                    out=fqt_lo,
                    in_=fqt_lo_d[:, v * QS:(v + 1) * QS].rearrange("(k p) q -> p k q", p=P))
            else:
                fqt_sb = fqt_pool.tile([P, NK, QS], mmdt)
                fqt_r = fqt_d[:, v * QS:(v + 1) * QS].rearrange(
                    "(k p) q -> p k q", p=P)
                for k in range(NK):
                    nc.sync.dma_start(out=fqt_sb[:, k, :], in_=fqt_r[:, k, :])

            kqb = []
            for li in range(NLEV):
                n = nmx[li]
                if n == 0:
                    kqb.append(None)
                    continue
                t = kq_pool.tile([P, nmx[0]], f32, tag=f"kqb{li}")
                nc.gpsimd.dma_start(
                    out=t[:, :n],
                    in_=kq_d[li:li + 1, v * QS: v * QS + n].to_broadcast([P, n]))
                kqb.append(t)
            mt = {}
            for li in range(1, NLEV):
                w = nmx[li] - nmn[li]
                if nmx[li] > 0 and w > 0:
                    t = kq_pool.tile([P, w], f32, tag=f"madd{li}")
                    nc.gpsimd.dma_start(
                        out=t, in_=madd_d[v, li:li + 1, 0].to_broadcast([P, w]))
                    mt[li] = t

            neglm_t = [st_pool.tile([P, NB], f32, tag=f"nl{li}", name=f"nl{li}_{v}")
                       for li in range(NLEV)]
            den_t = [st_pool.tile([P, NB], f32, tag=f"dn{li}", name=f"dn{li}_{v}")
                     for li in range(NLEV)]
            pos_t = [st_pool.tile([P, NB], f32, tag=f"po{li}", name=f"po{li}_{v}")
                     for li in range(NLEV)]

            for c in range(NB):
                ps = psum_pool.tile([P, QS], f32)
                ngr = QS // 512
                if two_pass:
                    passes = [(ft_hi, fqt_hi), (ft_hi, fqt_lo), (ft_lo, fqt_hi)]
                    for pi, (lt, rt) in enumerate(passes):
                        for k in range(NK):
                            for g in range(ngr):
                                gs = slice(g * 512, (g + 1) * 512)
                                nc.tensor.matmul(
                                    ps[:, gs],
                                    lt[:, k, c * P:(c + 1) * P],
                                    rt[:, k, gs],
                                    start=(pi == 0 and k == 0),
                                    stop=(pi == len(passes) - 1 and k == NK - 1))
                else:
                    # k outer / group inner: both 512-wide groups reuse the
                    # same stationary weight load
                    for k in range(NK):
                        for g in range(ngr):
                            gs = slice(g * 512, (g + 1) * 512)
                            nc.tensor.matmul(
                                ps[:, gs],
                                ft_sb[:, k, c * P:(c + 1) * P],
                                fqt_sb[:, k, gs],
                                start=(k == 0), stop=(k == NK - 1))

                for li in range(3):
                    n = nmx[li]
                    if n == 0:
                        continue
                    if li in mt:
                        nc.vector.tensor_add(
                            ps[:, nmn[li]:nmx[li]], ps[:, nmn[li]:nmx[li]], mt[li])
                    nc.vector.tensor_reduce(
                        neglm_t[li][:, c:c + 1], ps[:, :n],
                        axis=mybir.AxisListType.X, op=mybir.AluOpType.max,
                        negate=True)
                    e_scr = scr_pool.tile([P, nmx[0]], f32, tag="escr")
                    nc.scalar.activation(
                        e_scr[:, :n], ps[:, :n],
                        mybir.ActivationFunctionType.Exp,
                        bias=neglm_t[li][:, c:c + 1], scale=1.0,
                        accum_out=den_t[li][:, c:c + 1])
                    m_scr = scr_pool.tile([P, nmx[0]], f32, tag="mscr")
                    nc.vector.scalar_tensor_tensor(
                        out=m_scr[:, :n], in0=kqb[li][:, :n],
                        scalar=ka_sb[:, li, c:c + 1], in1=ps[:, :n],
                        op0=mybir.AluOpType.is_equal, op1=mybir.AluOpType.mult,
                        accum_out=pos_t[li][:, c:c + 1])

            for li in range(NLEV):
                for si, t in ((0, neglm_t[li]), (1, den_t[li]), (2, pos_t[li])):
                    nc.sync.dma_start(out=stats_d[v, li, si], in_=t)

    nc.compile()
    return nc


# ----------------------------------------------------------------- host prep
def _prepare(features, labels, features_queue, labels_queue):
    """Host-side: masks, balanced shard assignment, per-core input arrays."""
    B, D = features.shape
    Q = features_queue.shape[0]
    S = NCORES * NVS
    QS_SHARD = Q // S
    NB = B // P
    NLEV = 3

    levels = _host_masks(labels, labels_queue)

    # lifetime = last level at which a queue column is active (1..3)
    life = np.ones(Q, np.int64)
    for li in (1, 2):
        life += levels[li]["queue_active"].astype(np.int64)
    order_cols = np.argsort(-life, kind="stable")
    perm = order_cols.reshape(QS_SHARD, S).T  # [S, QS_SHARD]: shard s -> cols

    n_per_shard = np.zeros((S, NLEV), np.int64)
    n_per_shard[:, 0] = QS_SHARD
    for li in (1, 2):
        n_per_shard[:, li] = levels[li]["queue_active"][perm].sum(axis=1)
    nmx = [int(n_per_shard[:, li].max()) for li in range(NLEV)]
    nmn = [int(n_per_shard[:, li].min()) for li in range(NLEV)]
    assert nmx[0] == nmn[0] == QS_SHARD
    for li in range(1, NLEV):
        assert nmx[li] - nmn[li] <= 1, (nmx, nmn)

    # ---- per-core input arrays
    ftS = np.ascontiguousarray((features / TEMP).T)  # [D, B]
    fqT = np.ascontiguousarray(features_queue.T)     # [D, Q]

    ka_r = np.empty((NLEV, P, NB), np.float32)
    for li in range(NLEV):
        ka_r[li] = levels[li]["ka"].astype(np.float32).reshape(NB, P).T

    in_maps = []
    for c in range(NCORES):
        cols = perm[c * NVS:(c + 1) * NVS].reshape(-1)  # [CQ]
        fqt_c = np.ascontiguousarray(fqT[:, cols])
        kq_c = np.empty((NLEV, NVS * QS_SHARD), np.float32)
        for li in range(NLEV):
            kq_c[li] = np.where(
                levels[li]["queue_active"][cols],
                levels[li]["kq"][cols].astype(np.float32), np.float32(-1.0))
        madd_c = np.zeros((NVS, NLEV, 1), np.float32)
        for v in range(NVS):
            s = c * NVS + v
            for li in range(1, NLEV):
                if nmx[li] - nmn[li] > 0:
                    # mask the single boundary column if dead for this shard
                    madd_c[v, li, 0] = (
                        np.float32(MASK_VAL)
                        if n_per_shard[s, li] < nmx[li] else np.float32(0.0))
        m = {"kq": kq_c, "ka": ka_r, "madd": madd_c}
        if MM_MODE == "bf16x3":
            import ml_dtypes
            bf = ml_dtypes.bfloat16
            ft_hi = ftS.astype(bf)
            ft_lo = (ftS - ft_hi.astype(np.float32)).astype(bf)
            fq_hi = fqt_c.astype(bf)
            fq_lo = (fqt_c - fq_hi.astype(np.float32)).astype(bf)
            m.update(ft_hi=ft_hi, ft_lo=ft_lo, fqt_hi=fq_hi, fqt_lo=fq_lo)
        else:
            m.update(ft=ftS, fqt=fqt_c)
        in_maps.append(m)

    return dict(in_maps=in_maps, levels=levels, perm=perm,
                n_per_shard=n_per_shard, nmx=nmx, nmn=nmn,
                B=B, D=D, Q=Q, S=S, QS_SHARD=QS_SHARD, NB=NB, NLEV=NLEV)


# -------------------------------------------------------------------- kernel
def kernel(features, labels, features_queue, labels_queue):
    t0 = time.time()
    features = np.asarray(features, dtype=np.float32)
    features_queue = np.asarray(features_queue, dtype=np.float32)
    labels = np.asarray(labels)
    labels_queue = np.asarray(labels_queue)

    prep = _prepare(features, labels, features_queue, labels_queue)
    in_maps = prep["in_maps"]
    levels = prep["levels"]
    n_per_shard = prep["n_per_shard"]
    nmx, nmn = prep["nmx"], prep["nmn"]
    B, D = prep["B"], prep["D"]
    S, QS_SHARD = prep["S"], prep["QS_SHARD"]
    NLEV = prep["NLEV"]
    t_prep = time.time() - t0

    # ---- build + run device program
    t0 = time.time()
    nc = _build_program(D, B, NVS * QS_SHARD, nmx, nmn, MM_MODE)
    t_build = time.time() - t0

    t0 = time.time()
    br = run_bass_kernel_spmd(nc, in_maps, core_ids=list(range(NCORES)))
    t_run = time.time() - t0

    LAST_RUN.clear()
    LAST_RUN.update(
        exec_time_ns=br.exec_time_ns,
        mean_exec_time_ns=getattr(br, "mean_exec_time_ns", None),
        t_prep=t_prep, t_build=t_build, t_run=t_run,
        profile_json=br.profile_json,
        instructions_and_trace=br.instructions_and_trace,
        nmx=nmx, nmn=nmn)

    # ---- host merge (float64)
    t0 = time.time()
    # stats[c]: [NVS, NLEV, 3, P, NB] -> per shard arrays [B]
    neg_lm = np.empty((S, NLEV, B), np.float64)
    den = np.empty((S, NLEV, B), np.float64)
    pos = np.empty((S, NLEV, B), np.float64)
    for c in range(NCORES):
        st = br.results[c]["stats"]  # [NVS, NLEV, 3, P, NB]
        for v in range(NVS):
            s = c * NVS + v
            for li in range(NLEV):
                neg_lm[s, li] = st[v, li, 0].T.reshape(-1)
                den[s, li] = st[v, li, 1].T.reshape(-1)
                pos[s, li] = st[v, li, 2].T.reshape(-1)

    cum = 0.0
    max_lower = -np.inf
    for li in range(NLEV):
        l = li + 1
        cnt = levels[li]["cnt"].astype(np.float64)
        valid = n_per_shard[:, li] > 0  # shards with any columns at this level
        lm_s = -neg_lm[valid, li]      # [S', B]
        den_s = den[valid, li]
        pos_s = pos[valid, li]
        if lm_s.shape[0] == 0:
            layer_loss = 0.0
        else:
            lm = lm_s.max(axis=0)
            dtot = (den_s * np.exp(lm_s - lm[None, :])).sum(axis=0)
            ptot = pos_s.sum(axis=0)
            with np.errstate(divide="ignore", invalid="ignore"):
                mean = (ptot - cnt * (lm + np.log(dtot))) / (cnt + 1e-12)
            mean = np.where(cnt > 0, mean, 0.0)
            loss_i = -(TEMP / BASE_TEMP) * mean
            num = float((cnt > 0).sum())
            layer_loss = float(loss_i.sum() / (num + 1e-12))
        layer_loss = max(max_lower, layer_loss)
        cum = cum + (2.0 ** (1.0 / l)) * layer_loss
        max_lower = max(max_lower, layer_loss)

    LAST_RUN["t_merge"] = time.time() - t0
    return np.float32(cum)


# revision 22
# speedup vs baseline: 2.0814x; 1.3942x over previous
"""HMLC loss kernel for 8 Trainium2 NeuronCores (Bass/Tile).

Strategy (queue-sharded data parallelism):
  * All mask/dedup/queue-evolution logic in the reference depends ONLY on the
    integer labels -> computed exactly on host (numpy).
  * The queue (32768 cols) is split into 32 shards (8 cores x 4 vshards).
    Within each shard, columns are ordered by "lifetime" (the last level at
    which the column is still active), so the active set at every level is a
    prefix. The assignment is round-robin over the lifetime-sorted global
    column order, so prefix lengths differ by at most 1 across shards and a
    single compiled SPMD program (prefix bounds baked as max over shards)
    serves all cores; a <=1-column-wide additive -6e4 mask (per-core DATA)
    handles the remainder.
  * Device per (vshard, anchor-chunk): PE computes sim = (f/TEMP) @ fq_shard.T
    into PSUM [128,1024]; per level the stats are
        neg_lm  = -max(sim[:, :n])                  (VectorE tensor_reduce)
        denom   = sum exp(sim[:, :n] + neg_lm)      (ScalarE activation+accum)
        possum  = sum (kq==ka) * sim[:, :n]         (VectorE scalar_tensor_tensor+accum)
  * Host merges the 32 shards per level (online softmax) in float64 and runs
    the scalar hmce chain.
"""

import os
import sys
import time
from contextlib import ExitStack

if "/opt/trn_rl_repo" not in sys.path:
    sys.path.insert(0, "/opt/trn_rl_repo")

import numpy as np

import concourse.bass as bass  # noqa: E402
import concourse.bacc as bacc  # noqa: E402
import concourse.tile as tile  # noqa: E402
from concourse import mybir  # noqa: E402
from concourse.bass_utils import run_bass_kernel_spmd  # noqa: E402

TEMP = 0.07
BASE_TEMP = 0.07
NCORES = 8
NVS = 4          # vshards per core
P = 128          # partitions
MASK_VAL = -60000.0
# |sim| <= (1/TEMP) since features are L2-normalized -> a constant softmax
# shift is numerically safe and removes the per-row reduce_max entirely
CBIAS = 15.0

# matmul precision mode: "f32" (exact, 4 cyc/row), "f32r" (1 cyc/row),
# "bf16x3" (hi/lo split, 3 passes, 1 cyc/row each)
MM_MODE = os.environ.get("HMLC_MM_MODE", "f32r")

# populated by kernel() for test harness introspection
LAST_RUN = {}


# ---------------------------------------------------------------- host masks
def _host_masks(labels, labels_queue):
    """Exact replication of the reference's label-only mask evolution."""
    B, L = labels.shape
    Q = labels_queue.shape[0]
    base = int(max(labels.max(), labels_queue.max())) + 1
    pw = base ** np.arange(L - 1, -1, -1)

    anchor_active = np.ones(B, bool)
    queue_active = np.ones(Q, bool)
    order = np.arange(B)

    levels = []
    for l in range(1, L):
        ncols = L - l
        w = (pw * (np.arange(L) < ncols)).astype(np.int64)
        ka = labels.astype(np.int64) @ w
        kq = labels_queue.astype(np.int64) @ w
        maxk = int(max(ka.max(), kq.max())) + 1
        bc = np.bincount(kq[queue_active], minlength=maxk)
        cnt = np.where(anchor_active, bc[ka], 0)
        pres = np.zeros(maxk, bool)
        pres[ka[anchor_active]] = True
        newmatch = queue_active & pres[kq]
        levels.append(dict(
            ka=ka.copy(), kq=kq.copy(),
            queue_active=queue_active.copy(),
            cnt=cnt.copy(),
        ))
        same = (ka[:, None] == ka[None, :]) & anchor_active[:, None] & anchor_active[None, :]
        max_ord = np.max(np.where(same, order[None, :], -1), axis=1)
        kept = anchor_active & (order == max_ord)
        rank = (kept[None, :] & (ka[None, :] < ka[:, None])).sum(1)
        order = np.where(kept, rank, -1)
        anchor_active = kept
        queue_active = queue_active & ~newmatch
    return levels


# ------------------------------------------------------------ device program
def _build_program(D, B, CQ, nmx, nmn, mm_mode):
    NLEV = 3
    f32 = mybir.dt.float32
    NB = B // P       # anchor chunks
    NK = D // P       # contraction chunks
    QS = CQ // NVS    # vshard width

    nc = bacc.Bacc("TRN2", target_bir_lowering=False, debug=False)

    two_pass = mm_mode == "bf16x3"
    if two_pass:
        bf16 = mybir.dt.bfloat16
        ft_hi_d = nc.dram_tensor("ft_hi", [D, B], bf16, kind="ExternalInput").ap()
        ft_lo_d = nc.dram_tensor("ft_lo", [D, B], bf16, kind="ExternalInput").ap()
        fqt_hi_d = nc.dram_tensor("fqt_hi", [D, CQ], bf16, kind="ExternalInput").ap()
        fqt_lo_d = nc.dram_tensor("fqt_lo", [D, CQ], bf16, kind="ExternalInput").ap()
    else:
        mmdt = mybir.dt.float32r if mm_mode == "f32r" else f32
        ft_d = nc.dram_tensor("ft", [D, B], mmdt, kind="ExternalInput").ap()
        fqt_d = nc.dram_tensor("fqt", [D, CQ], mmdt, kind="ExternalInput").ap()
    kq_d = nc.dram_tensor("kq", [NLEV, CQ], f32, kind="ExternalInput").ap()
    ka_d = nc.dram_tensor("ka", [NLEV, P, NB], f32, kind="ExternalInput").ap()
    madd_d = nc.dram_tensor("madd", [NVS, NLEV, 1], f32, kind="ExternalInput").ap()
    stats_d = nc.dram_tensor(
        "stats", [NVS, NLEV, 3, P, NB], f32, kind="ExternalOutput").ap()

    with tile.TileContext(nc) as tc, ExitStack() as ctx:
        const_pool = ctx.enter_context(tc.tile_pool(name="const", bufs=1))
        fqt_pool = ctx.enter_context(tc.tile_pool(name="fqt", bufs=2))
        kq_pool = ctx.enter_context(tc.tile_pool(name="kqb", bufs=2))
        scr_pool = ctx.enter_context(tc.tile_pool(name="scr", bufs=4))
        st_pool = ctx.enter_context(tc.tile_pool(name="st", bufs=2))
        psum_pool = ctx.enter_context(tc.tile_pool(name="ps", bufs=4, space="PSUM"))

        if two_pass:
            ft_hi = const_pool.tile([P, NK, B], bf16)
            nc.sync.dma_start(out=ft_hi, in_=ft_hi_d.rearrange("(k p) b -> p k b", p=P))
            ft_lo = const_pool.tile([P, NK, B], bf16)
            nc.sync.dma_start(out=ft_lo, in_=ft_lo_d.rearrange("(k p) b -> p k b", p=P))
        else:
            ft_sb = const_pool.tile([P, NK, B], mmdt)
            ft_r = ft_d.rearrange("(k p) b -> p k b", p=P)
            for k in range(NK):
                nc.sync.dma_start(out=ft_sb[:, k, :], in_=ft_r[:, k, :])
        ka_sb = const_pool.tile([P, NLEV, NB], f32)
        nc.sync.dma_start(out=ka_sb, in_=ka_d.rearrange("l p c -> p l c"))
        cbias_sb = const_pool.tile([P, 1], f32)
        nc.vector.memset(cbias_sb, -CBIAS)

        for v in range(NVS):
            if two_pass:
                fqt_hi = fqt_pool.tile([P, NK, QS], bf16, tag="fqt_hi")
                nc.sync.dma_start(
                    out=fqt_hi,
                    in_=fqt_hi_d[:, v * QS:(v + 1) * QS].rearrange("(k p) q -> p k q", p=P))
                fqt_lo = fqt_pool.tile([P, NK, QS], bf16, tag="fqt_lo")
                nc.sync.dma_start(
                    out=fqt_lo,
                    in_=fqt_lo_d[:, v * QS:(v + 1) * QS].rearrange("(k p) q -> p k q", p=P))
            else:
                fqt_sb = fqt_pool.tile([P, NK, QS], mmdt)
                fqt_r = fqt_d[:, v * QS:(v + 1) * QS].rearrange(
                    "(k p) q -> p k q", p=P)
                for k in range(NK):
                    nc.sync.dma_start(out=fqt_sb[:, k, :], in_=fqt_r[:, k, :])

            kqb = []
            for li in range(NLEV):
                n = nmx[li]
                if n == 0:
                    kqb.append(None)
                    continue
                t = kq_pool.tile([P, nmx[0]], f32, tag=f"kqb{li}")
                nc.gpsimd.dma_start(
                    out=t[:, :n],
                    in_=kq_d[li:li + 1, v * QS: v * QS + n].to_broadcast([P, n]))
                kqb.append(t)
            mt = {}
            for li in range(1, NLEV):
                w = nmx[li] - nmn[li]
                if nmx[li] > 0 and w > 0:
                    t = kq_pool.tile([P, w], f32, tag=f"madd{li}")
                    nc.gpsimd.dma_start(
                        out=t, in_=madd_d[v, li:li + 1, 0].to_broadcast([P, w]))
                    mt[li] = t

            den_t = [st_pool.tile([P, NB], f32, tag=f"dn{li}", name=f"dn{li}_{v}")
                     for li in range(NLEV)]
            pos_t = [st_pool.tile([P, NB], f32, tag=f"po{li}", name=f"po{li}_{v}")
                     for li in range(NLEV)]

            for c in range(NB):
                ps = psum_pool.tile([P, QS], f32)
                ngr = QS // 512
                if two_pass:
                    passes = [(ft_hi, fqt_hi), (ft_hi, fqt_lo), (ft_lo, fqt_hi)]
                    for pi, (lt, rt) in enumerate(passes):
                        for k in range(NK):
                            for g in range(ngr):
                                gs = slice(g * 512, (g + 1) * 512)
                                nc.tensor.matmul(
                                    ps[:, gs],
                                    lt[:, k, c * P:(c + 1) * P],
                                    rt[:, k, gs],
                                    start=(pi == 0 and k == 0),
                                    stop=(pi == len(passes) - 1 and k == NK - 1))
                else:
                    # k outer / group inner: both 512-wide groups reuse the
                    # same stationary weight load
                    for k in range(NK):
                        for g in range(ngr):
                            gs = slice(g * 512, (g + 1) * 512)
                            nc.tensor.matmul(
                                ps[:, gs],
                                ft_sb[:, k, c * P:(c + 1) * P],
                                fqt_sb[:, k, gs],
                                start=(k == 0), stop=(k == NK - 1))

                for li in range(3):
                    n = nmx[li]
                    if n == 0:
                        continue
                    if li in mt:
                        nc.vector.tensor_add(
                            ps[:, nmn[li]:nmx[li]], ps[:, nmn[li]:nmx[li]], mt[li])
                    e_scr = scr_pool.tile([P, nmx[0]], f32, tag="escr")
                    nc.scalar.activation(
                        e_scr[:, :n], ps[:, :n],
                        mybir.ActivationFunctionType.Exp,
                        bias=cbias_sb[:, 0:1], scale=1.0,
                        accum_out=den_t[li][:, c:c + 1])
                    m_scr = scr_pool.tile([P, nmx[0]], f32, tag="mscr")
                    nc.vector.scalar_tensor_tensor(
                        out=m_scr[:, :n], in0=kqb[li][:, :n],
                        scalar=ka_sb[:, li, c:c + 1], in1=ps[:, :n],
                        op0=mybir.AluOpType.is_equal, op1=mybir.AluOpType.mult,
                        accum_out=pos_t[li][:, c:c + 1])

            for li in range(NLEV):
                for si, t in ((1, den_t[li]), (2, pos_t[li])):
                    nc.sync.dma_start(out=stats_d[v, li, si], in_=t)

    nc.compile()
    return nc


# ----------------------------------------------------------------- host prep
def _prepare(features, labels, features_queue, labels_queue):
    """Host-side: masks, balanced shard assignment, per-core input arrays."""
    B, D = features.shape
    Q = features_queue.shape[0]
    S = NCORES * NVS
    QS_SHARD = Q // S
    NB = B // P
    NLEV = 3

    levels = _host_masks(labels, labels_queue)

    # lifetime = last level at which a queue column is active (1..3)
    life = np.ones(Q, np.int64)
    for li in (1, 2):
        life += levels[li]["queue_active"].astype(np.int64)
    order_cols = np.argsort(-life, kind="stable")
    perm = order_cols.reshape(QS_SHARD, S).T  # [S, QS_SHARD]: shard s -> cols

    n_per_shard = np.zeros((S, NLEV), np.int64)
    n_per_shard[:, 0] = QS_SHARD
    for li in (1, 2):
        n_per_shard[:, li] = levels[li]["queue_active"][perm].sum(axis=1)
    nmx = [int(n_per_shard[:, li].max()) for li in range(NLEV)]
    nmn = [int(n_per_shard[:, li].min()) for li in range(NLEV)]
    assert nmx[0] == nmn[0] == QS_SHARD
    for li in range(1, NLEV):
        assert nmx[li] - nmn[li] <= 1, (nmx, nmn)

    # ---- per-core input arrays
    ftS = np.ascontiguousarray((features / TEMP).T)  # [D, B]
    fqT = np.ascontiguousarray(features_queue.T)     # [D, Q]

    ka_r = np.empty((NLEV, P, NB), np.float32)
    for li in range(NLEV):
        ka_r[li] = levels[li]["ka"].astype(np.float32).reshape(NB, P).T

    in_maps = []
    for c in range(NCORES):
        cols = perm[c * NVS:(c + 1) * NVS].reshape(-1)  # [CQ]
        fqt_c = np.ascontiguousarray(fqT[:, cols])
        kq_c = np.empty((NLEV, NVS * QS_SHARD), np.float32)
        for li in range(NLEV):
            kq_c[li] = np.where(
                levels[li]["queue_active"][cols],
                levels[li]["kq"][cols].astype(np.float32), np.float32(-1.0))
        madd_c = np.zeros((NVS, NLEV, 1), np.float32)
        for v in range(NVS):
            s = c * NVS + v
            for li in range(1, NLEV):
                if nmx[li] - nmn[li] > 0:
                    # mask the single boundary column if dead for this shard
                    madd_c[v, li, 0] = (
                        np.float32(MASK_VAL)
                        if n_per_shard[s, li] < nmx[li] else np.float32(0.0))
        m = {"kq": kq_c, "ka": ka_r, "madd": madd_c}
        if MM_MODE == "bf16x3":
            import ml_dtypes
            bf = ml_dtypes.bfloat16
            ft_hi = ftS.astype(bf)
            ft_lo = (ftS - ft_hi.astype(np.float32)).astype(bf)
            fq_hi = fqt_c.astype(bf)
            fq_lo = (fqt_c - fq_hi.astype(np.float32)).astype(bf)
            m.update(ft_hi=ft_hi, ft_lo=ft_lo, fqt_hi=fq_hi, fqt_lo=fq_lo)
        else:
            m.update(ft=ftS, fqt=fqt_c)
        in_maps.append(m)

    return dict(in_maps=in_maps, levels=levels, perm=perm,
                n_per_shard=n_per_shard, nmx=nmx, nmn=nmn,
                B=B, D=D, Q=Q, S=S, QS_SHARD=QS_SHARD, NB=NB, NLEV=NLEV)


# -------------------------------------------------------------------- kernel
def kernel(features, labels, features_queue, labels_queue):
    t0 = time.time()
    features = np.asarray(features, dtype=np.float32)
    features_queue = np.asarray(features_queue, dtype=np.float32)
    labels = np.asarray(labels)
    labels_queue = np.asarray(labels_queue)

    prep = _prepare(features, labels, features_queue, labels_queue)
    in_maps = prep["in_maps"]
    levels = prep["levels"]
    n_per_shard = prep["n_per_shard"]
    nmx, nmn = prep["nmx"], prep["nmn"]
    B, D = prep["B"], prep["D"]
    S, QS_SHARD = prep["S"], prep["QS_SHARD"]
    NLEV = prep["NLEV"]
    t_prep = time.time() - t0

    # ---- build + run device program
    t0 = time.time()
    nc = _build_program(D, B, NVS * QS_SHARD, nmx, nmn, MM_MODE)
    t_build = time.time() - t0

    t0 = time.time()
    br = run_bass_kernel_spmd(nc, in_maps, core_ids=list(range(NCORES)))
    t_run = time.time() - t0

    LAST_RUN.clear()
    LAST_RUN.update(
        exec_time_ns=br.exec_time_ns,
        mean_exec_time_ns=getattr(br, "mean_exec_time_ns", None),
        t_prep=t_prep, t_build=t_build, t_run=t_run,
        profile_json=br.profile_json,
        instructions_and_trace=br.instructions_and_trace,
        nmx=nmx, nmn=nmn)

    # ---- host merge (float64)
    t0 = time.time()
    # stats[c]: [NVS, NLEV, 3, P, NB] -> per shard arrays [B]
    neg_lm = np.empty((S, NLEV, B), np.float64)
    den = np.empty((S, NLEV, B), np.float64)
    pos = np.empty((S, NLEV, B), np.float64)
    for c in range(NCORES):
        st = br.results[c]["stats"]  # [NVS, NLEV, 3, P, NB]
        for v in range(NVS):
            s = c * NVS + v
            for li in range(NLEV):
                neg_lm[s, li] = -CBIAS  # constant softmax shift
                den[s, li] = st[v, li, 1].T.reshape(-1)
                pos[s, li] = st[v, li, 2].T.reshape(-1)

    cum = 0.0
    max_lower = -np.inf
    for li in range(NLEV):
        l = li + 1
        cnt = levels[li]["cnt"].astype(np.float64)
        valid = n_per_shard[:, li] > 0  # shards with any columns at this level
        lm_s = -neg_lm[valid, li]      # [S', B]
        den_s = den[valid, li]
        pos_s = pos[valid, li]
        if lm_s.shape[0] == 0:
            layer_loss = 0.0
        else:
            lm = lm_s.max(axis=0)
            dtot = (den_s * np.exp(lm_s - lm[None, :])).sum(axis=0)
            ptot = pos_s.sum(axis=0)
            with np.errstate(divide="ignore", invalid="ignore"):
                mean = (ptot - cnt * (lm + np.log(dtot))) / (cnt + 1e-12)
            mean = np.where(cnt > 0, mean, 0.0)
            loss_i = -(TEMP / BASE_TEMP) * mean
            num = float((cnt > 0).sum())
            layer_loss = float(loss_i.sum() / (num + 1e-12))
        layer_loss = max(max_lower, layer_loss)
        cum = cum + (2.0 ** (1.0 / l)) * layer_loss
        max_lower = max(max_lower, layer_loss)

    LAST_RUN["t_merge"] = time.time() - t0
    return np.float32(cum)


# revision 25
# speedup vs baseline: 2.0962x; 1.0071x over previous
"""HMLC loss kernel for 8 Trainium2 NeuronCores (Bass/Tile).

Strategy (queue-sharded data parallelism):
  * All mask/dedup/queue-evolution logic in the reference depends ONLY on the
    integer labels -> computed exactly on host (numpy).
  * The queue (32768 cols) is split into 32 shards (8 cores x 4 vshards).
    Within each shard, columns are ordered by "lifetime" (the last level at
    which the column is still active), so the active set at every level is a
    prefix. The assignment is round-robin over the lifetime-sorted global
    column order, so prefix lengths differ by at most 1 across shards and a
    single compiled SPMD program (prefix bounds baked as max over shards)
    serves all cores; a <=1-column-wide additive -6e4 mask (per-core DATA)
    handles the remainder.
  * Device per (vshard, anchor-chunk): PE computes sim = (f/TEMP) @ fq_shard.T
    into PSUM [128,1024]; per level the stats are
        neg_lm  = -max(sim[:, :n])                  (VectorE tensor_reduce)
        denom   = sum exp(sim[:, :n] + neg_lm)      (ScalarE activation+accum)
        possum  = sum (kq==ka) * sim[:, :n]         (VectorE scalar_tensor_tensor+accum)
  * Host merges the 32 shards per level (online softmax) in float64 and runs
    the scalar hmce chain.
"""

import os
import sys
import time
from contextlib import ExitStack

if "/opt/trn_rl_repo" not in sys.path:
    sys.path.insert(0, "/opt/trn_rl_repo")

import numpy as np

import concourse.bass as bass  # noqa: E402
import concourse.bacc as bacc  # noqa: E402
import concourse.tile as tile  # noqa: E402
from concourse import mybir  # noqa: E402
from concourse.bass_utils import run_bass_kernel_spmd  # noqa: E402

TEMP = 0.07
BASE_TEMP = 0.07
NCORES = 8
NVS = 4          # vshards per core
P = 128          # partitions
MASK_VAL = -60000.0
# |sim| <= (1/TEMP) since features are L2-normalized -> a constant softmax
# shift is numerically safe and removes the per-row reduce_max entirely
CBIAS = 15.0

# matmul precision mode: "f32" (exact, 4 cyc/row), "f32r" (1 cyc/row),
# "bf16x3" (hi/lo split, 3 passes, 1 cyc/row each)
MM_MODE = os.environ.get("HMLC_MM_MODE", "f32r")

# populated by kernel() for test harness introspection
LAST_RUN = {}


# ---------------------------------------------------------------- host masks
def _host_masks(labels, labels_queue):
    """Exact replication of the reference's label-only mask evolution."""
    B, L = labels.shape
    Q = labels_queue.shape[0]
    base = int(max(labels.max(), labels_queue.max())) + 1
    pw = base ** np.arange(L - 1, -1, -1)

    anchor_active = np.ones(B, bool)
    queue_active = np.ones(Q, bool)
    order = np.arange(B)

    levels = []
    for l in range(1, L):
        ncols = L - l
        w = (pw * (np.arange(L) < ncols)).astype(np.int64)
        ka = labels.astype(np.int64) @ w
        kq = labels_queue.astype(np.int64) @ w
        maxk = int(max(ka.max(), kq.max())) + 1
        bc = np.bincount(kq[queue_active], minlength=maxk)
        cnt = np.where(anchor_active, bc[ka], 0)
        pres = np.zeros(maxk, bool)
        pres[ka[anchor_active]] = True
        newmatch = queue_active & pres[kq]
        levels.append(dict(
            ka=ka.copy(), kq=kq.copy(),
            queue_active=queue_active.copy(),
            cnt=cnt.copy(),
        ))
        same = (ka[:, None] == ka[None, :]) & anchor_active[:, None] & anchor_active[None, :]
        max_ord = np.max(np.where(same, order[None, :], -1), axis=1)
        kept = anchor_active & (order == max_ord)
        rank = (kept[None, :] & (ka[None, :] < ka[:, None])).sum(1)
        order = np.where(kept, rank, -1)
        anchor_active = kept
        queue_active = queue_active & ~newmatch
    return levels


# ------------------------------------------------------------ device program
def _build_program(D, B, CQ, nmx, nmn, mm_mode):
    NLEV = 3
    f32 = mybir.dt.float32
    NB = B // P       # anchor chunks
    NK = D // P       # contraction chunks
    QS = CQ // NVS    # vshard width

    nc = bacc.Bacc("TRN2", target_bir_lowering=False, debug=False)

    two_pass = mm_mode == "bf16x3"
    if two_pass:
        bf16 = mybir.dt.bfloat16
        ft_hi_d = nc.dram_tensor("ft_hi", [D, B], bf16, kind="ExternalInput").ap()
        ft_lo_d = nc.dram_tensor("ft_lo", [D, B], bf16, kind="ExternalInput").ap()
        fqt_hi_d = nc.dram_tensor("fqt_hi", [D, CQ], bf16, kind="ExternalInput").ap()
        fqt_lo_d = nc.dram_tensor("fqt_lo", [D, CQ], bf16, kind="ExternalInput").ap()
    else:
        mmdt = mybir.dt.float32r if mm_mode == "f32r" else f32
        ft_d = nc.dram_tensor("ft", [D, B], mmdt, kind="ExternalInput").ap()
        fqt_d = nc.dram_tensor("fqt", [D, CQ], mmdt, kind="ExternalInput").ap()
    kq_d = nc.dram_tensor("kq", [NLEV, CQ], f32, kind="ExternalInput").ap()
    ka_d = nc.dram_tensor("ka", [NLEV, P, NB], f32, kind="ExternalInput").ap()
    madd_d = nc.dram_tensor("madd", [NVS, NLEV, 1], f32, kind="ExternalInput").ap()
    stats_d = nc.dram_tensor(
        "stats", [NVS, NLEV, 3, P, NB], f32, kind="ExternalOutput").ap()

    with tile.TileContext(nc) as tc, ExitStack() as ctx:
        const_pool = ctx.enter_context(tc.tile_pool(name="const", bufs=1))
        fqt_pool = ctx.enter_context(tc.tile_pool(name="fqt", bufs=2))
        kq_pool = ctx.enter_context(tc.tile_pool(name="kqb", bufs=2))
        scr_pool = ctx.enter_context(tc.tile_pool(name="scr", bufs=4))
        st_pool = ctx.enter_context(tc.tile_pool(name="st", bufs=2))
        psum_pool = ctx.enter_context(tc.tile_pool(name="ps", bufs=4, space="PSUM"))

        if two_pass:
            ft_hi = const_pool.tile([P, NK, B], bf16)
            nc.sync.dma_start(out=ft_hi, in_=ft_hi_d.rearrange("(k p) b -> p k b", p=P))
            ft_lo = const_pool.tile([P, NK, B], bf16)
            nc.sync.dma_start(out=ft_lo, in_=ft_lo_d.rearrange("(k p) b -> p k b", p=P))
        else:
            ft_sb = const_pool.tile([P, NK, B], mmdt)
            ft_r = ft_d.rearrange("(k p) b -> p k b", p=P)
            # ft DMAs are interleaved with the first vshard's fqt chunks below
            # so the first matmuls can start after ~one k-chunk of each
        ka_sb = const_pool.tile([P, NLEV, NB], f32)
        nc.gpsimd.dma_start(out=ka_sb, in_=ka_d.rearrange("l p c -> p l c"))
        cbias_sb = const_pool.tile([P, 1], f32)
        nc.vector.memset(cbias_sb, -CBIAS)

        for v in range(NVS):
            if two_pass:
                fqt_hi = fqt_pool.tile([P, NK, QS], bf16, tag="fqt_hi")
                nc.sync.dma_start(
                    out=fqt_hi,
                    in_=fqt_hi_d[:, v * QS:(v + 1) * QS].rearrange("(k p) q -> p k q", p=P))
                fqt_lo = fqt_pool.tile([P, NK, QS], bf16, tag="fqt_lo")
                nc.sync.dma_start(
                    out=fqt_lo,
                    in_=fqt_lo_d[:, v * QS:(v + 1) * QS].rearrange("(k p) q -> p k q", p=P))
            else:
                fqt_sb = fqt_pool.tile([P, NK, QS], mmdt)
                fqt_r = fqt_d[:, v * QS:(v + 1) * QS].rearrange(
                    "(k p) q -> p k q", p=P)
                for k in range(NK):
                    nc.sync.dma_start(out=fqt_sb[:, k, :], in_=fqt_r[:, k, :])
                    if v == 0:
                        nc.sync.dma_start(out=ft_sb[:, k, :], in_=ft_r[:, k, :])

            kqb = []
            for li in range(NLEV):
                n = nmx[li]
                if n == 0:
                    kqb.append(None)
                    continue
                t = kq_pool.tile([P, nmx[0]], f32, tag=f"kqb{li}")
                nc.gpsimd.dma_start(
                    out=t[:, :n],
                    in_=kq_d[li:li + 1, v * QS: v * QS + n].to_broadcast([P, n]))
                kqb.append(t)
            mt = {}
            for li in range(1, NLEV):
                w = nmx[li] - nmn[li]
                if nmx[li] > 0 and w > 0:
                    t = kq_pool.tile([P, w], f32, tag=f"madd{li}")
                    nc.gpsimd.dma_start(
                        out=t, in_=madd_d[v, li:li + 1, 0].to_broadcast([P, w]))
                    mt[li] = t

            den_t = [st_pool.tile([P, NB], f32, tag=f"dn{li}", name=f"dn{li}_{v}")
                     for li in range(NLEV)]
            pos_t = [st_pool.tile([P, NB], f32, tag=f"po{li}", name=f"po{li}_{v}")
                     for li in range(NLEV)]

            for c in range(NB):
                ps = psum_pool.tile([P, QS], f32)
                ngr = QS // 512
                if two_pass:
                    passes = [(ft_hi, fqt_hi), (ft_hi, fqt_lo), (ft_lo, fqt_hi)]
                    for pi, (lt, rt) in enumerate(passes):
                        for k in range(NK):
                            for g in range(ngr):
                                gs = slice(g * 512, (g + 1) * 512)
                                nc.tensor.matmul(
                                    ps[:, gs],
                                    lt[:, k, c * P:(c + 1) * P],
                                    rt[:, k, gs],
                                    start=(pi == 0 and k == 0),
                                    stop=(pi == len(passes) - 1 and k == NK - 1))
                else:
                    # k outer / group inner: both 512-wide groups reuse the
                    # same stationary weight load
                    for k in range(NK):
                        for g in range(ngr):
                            gs = slice(g * 512, (g + 1) * 512)
                            nc.tensor.matmul(
                                ps[:, gs],
                                ft_sb[:, k, c * P:(c + 1) * P],
                                fqt_sb[:, k, gs],
                                start=(k == 0), stop=(k == NK - 1))

                for li in range(3):
                    n = nmx[li]
                    if n == 0:
                        continue
                    if li in mt:
                        nc.vector.tensor_add(
                            ps[:, nmn[li]:nmx[li]], ps[:, nmn[li]:nmx[li]], mt[li])
                    e_scr = scr_pool.tile([P, nmx[0]], f32, tag="escr")
                    nc.scalar.activation(
                        e_scr[:, :n], ps[:, :n],
                        mybir.ActivationFunctionType.Exp,
                        bias=cbias_sb[:, 0:1], scale=1.0,
                        accum_out=den_t[li][:, c:c + 1])
                    m_scr = scr_pool.tile([P, nmx[0]], f32, tag="mscr")
                    nc.vector.scalar_tensor_tensor(
                        out=m_scr[:, :n], in0=kqb[li][:, :n],
                        scalar=ka_sb[:, li, c:c + 1], in1=ps[:, :n],
                        op0=mybir.AluOpType.is_equal, op1=mybir.AluOpType.mult,
                        accum_out=pos_t[li][:, c:c + 1])

            for li in range(NLEV):
                for si, t in ((1, den_t[li]), (2, pos_t[li])):
                    nc.sync.dma_start(out=stats_d[v, li, si], in_=t)

    nc.compile()
    return nc


# ----------------------------------------------------------------- host prep
def _prepare(features, labels, features_queue, labels_queue):
    """Host-side: masks, balanced shard assignment, per-core input arrays."""
    B, D = features.shape
    Q = features_queue.shape[0]
    S = NCORES * NVS
    QS_SHARD = Q // S
    NB = B // P
    NLEV = 3

    levels = _host_masks(labels, labels_queue)

    # lifetime = last level at which a queue column is active (1..3)
    life = np.ones(Q, np.int64)
    for li in (1, 2):
        life += levels[li]["queue_active"].astype(np.int64)
    order_cols = np.argsort(-life, kind="stable")
    perm = order_cols.reshape(QS_SHARD, S).T  # [S, QS_SHARD]: shard s -> cols

    n_per_shard = np.zeros((S, NLEV), np.int64)
    n_per_shard[:, 0] = QS_SHARD
    for li in (1, 2):
        n_per_shard[:, li] = levels[li]["queue_active"][perm].sum(axis=1)
    nmx = [int(n_per_shard[:, li].max()) for li in range(NLEV)]
    nmn = [int(n_per_shard[:, li].min()) for li in range(NLEV)]
    assert nmx[0] == nmn[0] == QS_SHARD
    for li in range(1, NLEV):
        assert nmx[li] - nmn[li] <= 1, (nmx, nmn)

    # ---- per-core input arrays
    ftS = np.ascontiguousarray((features / TEMP).T)  # [D, B]
    fqT = np.ascontiguousarray(features_queue.T)     # [D, Q]

    ka_r = np.empty((NLEV, P, NB), np.float32)
    for li in range(NLEV):
        ka_r[li] = levels[li]["ka"].astype(np.float32).reshape(NB, P).T

    in_maps = []
    for c in range(NCORES):
        cols = perm[c * NVS:(c + 1) * NVS].reshape(-1)  # [CQ]
        fqt_c = np.ascontiguousarray(fqT[:, cols])
        kq_c = np.empty((NLEV, NVS * QS_SHARD), np.float32)
        for li in range(NLEV):
            kq_c[li] = np.where(
                levels[li]["queue_active"][cols],
                levels[li]["kq"][cols].astype(np.float32), np.float32(-1.0))
        madd_c = np.zeros((NVS, NLEV, 1), np.float32)
        for v in range(NVS):
            s = c * NVS + v
            for li in range(1, NLEV):
                if nmx[li] - nmn[li] > 0:
                    # mask the single boundary column if dead for this shard
                    madd_c[v, li, 0] = (
                        np.float32(MASK_VAL)
                        if n_per_shard[s, li] < nmx[li] else np.float32(0.0))
        m = {"kq": kq_c, "ka": ka_r, "madd": madd_c}
        if MM_MODE == "bf16x3":
            import ml_dtypes
            bf = ml_dtypes.bfloat16
            ft_hi = ftS.astype(bf)
            ft_lo = (ftS - ft_hi.astype(np.float32)).astype(bf)
            fq_hi = fqt_c.astype(bf)
            fq_lo = (fqt_c - fq_hi.astype(np.float32)).astype(bf)
            m.update(ft_hi=ft_hi, ft_lo=ft_lo, fqt_hi=fq_hi, fqt_lo=fq_lo)
        else:
            m.update(ft=ftS, fqt=fqt_c)
        in_maps.append(m)

    return dict(in_maps=in_maps, levels=levels, perm=perm,
                n_per_shard=n_per_shard, nmx=nmx, nmn=nmn,
                B=B, D=D, Q=Q, S=S, QS_SHARD=QS_SHARD, NB=NB, NLEV=NLEV)


# -------------------------------------------------------------------- kernel
def kernel(features, labels, features_queue, labels_queue):
    t0 = time.time()
    features = np.asarray(features, dtype=np.float32)
    features_queue = np.asarray(features_queue, dtype=np.float32)
    labels = np.asarray(labels)
    labels_queue = np.asarray(labels_queue)

    prep = _prepare(features, labels, features_queue, labels_queue)
    in_maps = prep["in_maps"]
    levels = prep["levels"]
    n_per_shard = prep["n_per_shard"]
    nmx, nmn = prep["nmx"], prep["nmn"]
    B, D = prep["B"], prep["D"]
    S, QS_SHARD = prep["S"], prep["QS_SHARD"]
    NLEV = prep["NLEV"]
    t_prep = time.time() - t0

    # ---- build + run device program
    t0 = time.time()
    nc = _build_program(D, B, NVS * QS_SHARD, nmx, nmn, MM_MODE)
    t_build = time.time() - t0

    t0 = time.time()
    br = run_bass_kernel_spmd(nc, in_maps, core_ids=list(range(NCORES)))
    t_run = time.time() - t0

    LAST_RUN.clear()
    LAST_RUN.update(
        exec_time_ns=br.exec_time_ns,
        mean_exec_time_ns=getattr(br, "mean_exec_time_ns", None),
        t_prep=t_prep, t_build=t_build, t_run=t_run,
        profile_json=br.profile_json,
        instructions_and_trace=br.instructions_and_trace,
        nmx=nmx, nmn=nmn)

    # ---- host merge (float64)
    t0 = time.time()
    # stats[c]: [NVS, NLEV, 3, P, NB] -> per shard arrays [B]
    neg_lm = np.empty((S, NLEV, B), np.float64)
    den = np.empty((S, NLEV, B), np.float64)
    pos = np.empty((S, NLEV, B), np.float64)
    for c in range(NCORES):
        st = br.results[c]["stats"]  # [NVS, NLEV, 3, P, NB]
        for v in range(NVS):
            s = c * NVS + v
            for li in range(NLEV):
                neg_lm[s, li] = -CBIAS  # constant softmax shift
                den[s, li] = st[v, li, 1].T.reshape(-1)
                pos[s, li] = st[v, li, 2].T.reshape(-1)

    cum = 0.0
    max_lower = -np.inf
    for li in range(NLEV):
        l = li + 1
        cnt = levels[li]["cnt"].astype(np.float64)
        valid = n_per_shard[:, li] > 0  # shards with any columns at this level
        lm_s = -neg_lm[valid, li]      # [S', B]
        den_s = den[valid, li]
        pos_s = pos[valid, li]
        if lm_s.shape[0] == 0:
            layer_loss = 0.0
        else:
            lm = lm_s.max(axis=0)
            dtot = (den_s * np.exp(lm_s - lm[None, :])).sum(axis=0)
            ptot = pos_s.sum(axis=0)
            with np.errstate(divide="ignore", invalid="ignore"):
                mean = (ptot - cnt * (lm + np.log(dtot))) / (cnt + 1e-12)
            mean = np.where(cnt > 0, mean, 0.0)
            loss_i = -(TEMP / BASE_TEMP) * mean
            num = float((cnt > 0).sum())
            layer_loss = float(loss_i.sum() / (num + 1e-12))
        layer_loss = max(max_lower, layer_loss)
        cum = cum + (2.0 ** (1.0 / l)) * layer_loss
        max_lower = max(max_lower, layer_loss)

    LAST_RUN["t_merge"] = time.time() - t0
    return np.float32(cum)
